# revision 55
# baseline (speedup 1.0000x reference)
"""Causal self-attention (GPT-2 style) on 8 TRN2 NeuronCores.

Sharding: B=2 x H=12 -> 24 (batch, head) pairs; core c handles batch c//4
and heads [3*(c%4), 3*(c%4)+3). Each core computes QKV for its 3 heads,
causal attention (flash-style, scores^T layout), and a partial output
projection; the host sums the 4 per-batch partials and adds b_proj.

v2: ACT(exp)-bound design. The S^T matmuls run as K=64 row-packed pairs
(tile_position (0,0)/(64,0)) so two streams' score matmuls co-execute on
the PE; QKV and output-projection work is interleaved into the
attention slots as PE filler; softmax finalize is batched per stream
pair (one Ln+Exp on [2,512], one K=2 broadcast matmul via a [2,128]
selector). Per-head Q^T/K^T live in opposite partition halves
(h0/h1 packed; h2 duplicated) which also packs the projection
contraction to a full 128 rows. All persistent intermediates are
per-block tiles (single writer) so fused-phase dependency tracking
stays fine-grained.

Self-contained: builds the Bass program on first call, runs via
run_bass_kernel_spmd on cores 0-7.
"""
import numpy as np
import ml_dtypes

import concourse.bass as bass
import concourse.mybir as mybir
import concourse.tile as tile
from concourse.bass import ts
from concourse.vector_clock import ScopedClock
from concourse.bass_utils import run_bass_kernel_spmd

# ---------------------------------------------------------------------------
# Workaround for the container's walrus build, which rejects any instruction
# carrying more than ONE sync-wait command ("Too many sync wait commands").
# 1) patch the TileContext tail drain to funnel its wait-set through
#    single-wait NOPs on SP; 2) post-pass that moves excess on_wait entries
#    from any instruction onto single-wait NOPs inserted before it on the
#    same engine (engine stalls on the NOPs, then issues the instruction —
#    semantics preserved).
# ---------------------------------------------------------------------------
_WAIT_LIMIT = 1


def _patched_drain_and_barrier(self, tick_clock, wait_clock):
    nc = self.nc
    carrier = nc.sync.nop()
    wait_clock.add_sem_waits(carrier.ins, ScopedClock({None: tick_clock.global_clock}))
    si = carrier.ins.sync_info
    waits = list(si.on_wait) if si and si.on_wait else []
    if len(waits) > _WAIT_LIMIT:
        si.on_wait = waits[:_WAIT_LIMIT]
        for w in waits[_WAIT_LIMIT:]:
            n2 = nc.sync.nop()
            s2 = n2.ins.sync_info
            if s2 is None:
                n2.ins.sync_info = mybir.SyncInfo(on_wait=[w], on_update=[])
            else:
                s2.on_wait = [w]
    nc.sync.drain()
    nc.all_engine_barrier()
    popped = nc._tile_sem_poison_stack.pop()
    assert popped is self._sem_poison
    nc.clear_and_free_semaphores(list(self.sems.allocated().values()))
    nc.all_engine_barrier()


tile.TileContext._drain_and_barrier = _patched_drain_and_barrier


def _split_multi_waits(nc):
    n_inserted = 0
    for fn in nc.m.functions:
        for blk in fn.blocks:
            new_list = []
            changed = False
            for inst in blk.instructions:
                si = getattr(inst, "sync_info", None)
                waits = list(si.on_wait) if (si is not None and si.on_wait) else []
                if len(waits) > _WAIT_LIMIT:
                    extra = waits[: len(waits) - _WAIT_LIMIT]
                    keep = waits[len(waits) - _WAIT_LIMIT:]
                    for w in extra:
                        nop = mybir.InstNoOp(
                            name=f"wsplit-{n_inserted}",
                            sync_info=mybir.SyncInfo(on_wait=[w], on_update=[]),
                            bass_nofuse=True,
                            engine=inst.engine,
                        )
                        new_list.append(nop)
                        n_inserted += 1
                    si.on_wait = keep
                    changed = True
                new_list.append(inst)
            if changed:
                blk.instructions = new_list
    return n_inserted


# ---------------------------------------------------------------------------
# Problem constants (hardcoded per contract).
# ---------------------------------------------------------------------------
B, S, E, H = 2, 4096, 768, 12
D = 64           # head dim
HPC = 3          # heads per core
NCORES = 8
BF16 = mybir.dt.bfloat16
F32 = mybir.dt.float32
QB = 512         # q-block width
GK = 3           # k-tiles per exp group (3 PSUM banks per sp tile)
NQB = S // QB    # 8
NKT = S // 128   # 32 k-tiles

TRACE = False
LAST_EXEC_NS = None

_nc = {}


def _echunks(with_bias):
    # contraction chunks over the (augmented) feature dim
    ch = [(e * 128, 128) for e in range(6)]
    if with_bias:
        ch.append((768, 64))  # ones/bias row (+ zero padding)
    return ch


def _build_program(with_bias):
    nc = bass.Bass()
    EA = 832 if with_bias else 768
    ech = _echunks(with_bias)
    NE = len(ech)

    xT = nc.dram_tensor("xT", [EA, S], BF16, kind="ExternalInput")
    wqk = nc.dram_tensor("wqk", [EA, 2 * HPC * D], BF16, kind="ExternalInput")
    wv = nc.dram_tensor("wv", [EA, HPC * D], BF16, kind="ExternalInput")
    wpab = nc.dram_tensor("wpab", [128, E], BF16, kind="ExternalInput")
    wp2 = nc.dram_tensor("wp2", [D, E], BF16, kind="ExternalInput")
    tri = nc.dram_tensor("tri", [128, 128], BF16, kind="ExternalInput")
    bsel = nc.dram_tensor("bsel", [33, 128], BF16, kind="ExternalInput")
    y = nc.dram_tensor("y", [S, E], F32, kind="ExternalOutput")

    with tile.TileContext(nc) as tc:
        with (
            tc.tile_pool(name="wpool", bufs=1) as wpool,
            tc.tile_pool(name="per", bufs=1) as per,
            tc.tile_pool(name="xch", bufs=2) as xch,
            tc.tile_pool(name="sps", bufs=2, space="PSUM") as sps,
            tc.tile_pool(name="ops", bufs=2, space="PSUM") as ops,
            tc.tile_pool(name="asb", bufs=8) as asb,
            tc.tile_pool(name="nrm", bufs=2) as nrm,
            tc.tile_pool(name="ysb", bufs=2) as ysb,
        ):
            FQK = 2 * HPC * D
            xc_cur = {}

            def emit_x_dma(tb):
                xc = []
                for e, (r0, rn) in enumerate(ech):
                    t = xch.tile([rn, QB], BF16, name=f"xc{e}", tag=f"xc{e}")
                    nc.sync.dma_start(out=t, in_=xT[r0:r0 + rn, ts(tb, QB)])
                    xc.append(t)
                xc_cur[tb] = xc

            # --- weights to SBUF (critical-path first: wqk+x0 chunk
            # pairs interleaved so the first QK matmul can start after
            # two DMAs, not thirteen) ---
            bsel_sb = wpool.tile([33, 128], BF16, name="bsel_sb")
            nc.sync.dma_start(out=bsel_sb, in_=bsel[:, :])
            wqk_sb, wv_sb = [], []
            xc0 = []
            for e, (r0, rn) in enumerate(ech):
                t1 = wpool.tile([rn, FQK], BF16, name=f"wqk{e}")
                nc.sync.dma_start(out=t1, in_=wqk[r0:r0 + rn, :])
                wqk_sb.append(t1)
                t = xch.tile([rn, QB], BF16, name=f"xc{e}", tag=f"xc{e}")
                nc.sync.dma_start(out=t, in_=xT[r0:r0 + rn, 0:QB])
                xc0.append(t)
            xc_cur[0] = xc0
            emit_x_dma(1)
            for e, (r0, rn) in enumerate(ech):
                t2 = wpool.tile([rn, HPC * D], BF16, name=f"wv{e}")
                nc.sync.dma_start(out=t2, in_=wv[r0:r0 + rn, :])
                wv_sb.append(t2)
            # packed projection weights: wpab rows 0-63 = W_h0, 64-127 = W_h1
            wpab_sb = wpool.tile([128, E], BF16, name="wpab")
            nc.sync.dma_start(out=wpab_sb, in_=wpab[:, :])
            # h2 parity tiles: even q-blocks use rows 0-63 of ot2, odd 64-127
            wp2e = wpool.tile([128, E], BF16, name="wp2e")
            nc.sync.dma_start(out=wp2e[0:64, :], in_=wp2[:, :])
            nc.gpsimd.memset(wp2e[64:128, :], 0.0)
            wp2o = wpool.tile([128, E], BF16, name="wp2o")
            nc.gpsimd.memset(wp2o[0:64, :], 0.0)
            nc.sync.dma_start(out=wp2o[64:128, :], in_=wp2[:, :])
            tri_sb = wpool.tile([128, 128], BF16, name="tri_sb")
            nc.sync.dma_start(out=tri_sb, in_=tri[:, :])

            # --- persistent intermediates, per 512-token block ---
            # Feature-major Q^T/K^T: h0 in rows 0-63 + h1 in rows 64-127
            # (row-packed score matmuls); h2 duplicated into both halves.
            qtab = [per.tile([128, QB], BF16, name=f"qtab{j}") for j in range(NQB)]
            ktab = [per.tile([128, QB], BF16, name=f"ktab{j}") for j in range(NQB)]
            qt2 = [per.tile([128, QB], BF16, name=f"qt2_{j}") for j in range(NQB)]
            kt2 = [per.tile([128, QB], BF16, name=f"kt2_{j}") for j in range(NQB)]
            # vtok[h][tb]: token-major V with a ones column per k-tile:
            # cols [65s, 65s+64) = V rows of k-tile 4tb+s, col 65s+64 = 1.0
            vtok = [[per.tile([128, 65 * 4], BF16, name=f"vtok{h}_{j}")
                     for j in range(NQB)] for h in range(HPC)]
            for h in range(HPC):
                for j in range(NQB):
                    nc.vector.memset(vtok[h][j], 1.0)
            # normalized O^T per block: otab rows 0-63 = h0, 64-127 = h1;
            # ot2 rows 0-63 valid on even blocks, 64-127 on odd (other
            # half is garbage, masked by wp2e/wp2o zeros).
            otab = [per.tile([128, QB], BF16, name=f"otab{j}") for j in range(NQB)]
            ot2 = [per.tile([128, QB], BF16, name=f"ot2_{j}") for j in range(NQB)]
            for j in range(NQB):
                # the unused parity half must be zeros, not garbage:
                # 0 x NaN = NaN would poison the projection accumulation
                if j % 2 == 0:
                    nc.gpsimd.memset(ot2[j][64:128, :], 0.0)
                else:
                    nc.gpsimd.memset(ot2[j][0:64, :], 0.0)

            if True:
                # ACT spline-table preload: a throwaway exp during the
                # prologue pulls the ~2.7us ACT_TABLE_LOAD off phase 2's
                # critical path
                warm = nrm.tile([2, 128], F32, name="warm", tag="warm")
                nc.scalar.activation(warm, bsel_sb[0:2, :],
                                     mybir.ActivationFunctionType.Exp)
                def fill_psum():
                    # fillers borrow a score-pool slot (PSUM is fully
                    # subscribed: 2x3 sp banks + 2 otp banks)
                    return sps.tile([128, GK * QB], F32, name="fp", tag="sp")

                # prime the PE's HAM clock gate during the initial DMA
                # wait: ~36 dummy matmuls on the tiny bsel tile give
                # ~4us of continuous PE activity so the real prologue
                # matmuls run at 2.4GHz instead of 1.2
                pump = fill_psum()
                for _ in range(36):
                    nc.tensor.matmul(pump[:, 0:128], bsel_sb, bsel_sb,
                                     start=True, stop=True)

                def emit_qk_ftile(tb, f):
                    # out rows = 128 cols of wqk f-tile; f0=[q0|q1]->qtab,
                    # f1=[k0|k1]->ktab, f2=[q2|k2]->qt2/kt2 duplicated
                    xc = xc_cur[tb]
                    ps = fill_psum()[:, 0:QB]
                    for e in range(NE):
                        nc.tensor.matmul(ps, wqk_sb[e][:, ts(f, 128)], xc[e],
                                         start=(e == 0), stop=(e == NE - 1))
                    if f == 0:
                        nc.vector.tensor_copy(qtab[tb], ps)
                    elif f == 1:
                        nc.vector.tensor_copy(ktab[tb], ps)
                    else:
                        nc.vector.tensor_copy(qt2[tb][0:64, :], ps[0:64, :])
                        nc.vector.tensor_copy(qt2[tb][64:128, :], ps[0:64, :])
                        nc.vector.tensor_copy(kt2[tb][0:64, :], ps[64:128, :])
                        nc.vector.tensor_copy(kt2[tb][64:128, :], ps[64:128, :])

                def emit_v_stile(tb, st):
                    xc = xc_cur[tb]
                    vp = fill_psum()[:, 0:HPC * D]
                    for e in range(NE):
                        nc.tensor.matmul(vp, xc[e][:, ts(st, 128)], wv_sb[e],
                                         start=(e == 0), stop=(e == NE - 1))
                    for h in range(HPC):
                        nc.vector.tensor_copy(
                            vtok[h][tb][:, st * 65: st * 65 + 64],
                            vp[:, ts(h, D)])
                    if st == 3:
                        del xc_cur[tb]

                def qkv_units(tb):
                    u = [(1300, lambda tb=tb, f=f: emit_qk_ftile(tb, f))
                         for f in range(3)]
                    u += [(700, lambda tb=tb, st=st: emit_v_stile(tb, st))
                          for st in range(4)]
                    return u

                def qk01_units(tb):
                    return [(1300, lambda tb=tb, f=f: emit_qk_ftile(tb, f))
                            for f in range(2)]

                def rest_units(tb):
                    u = [(1300, lambda tb=tb: emit_qk_ftile(tb, 2))]
                    u += [(700, lambda tb=tb, st=st: emit_v_stile(tb, st))
                          for st in range(4)]
                    return u

                def emit_proj(tt):
                    Jb = tt // 4
                    wp2x = wp2e if (Jb % 2 == 0) else wp2o
                    yt = ysb.tile([128, E], F32, name="yt", tag="yt")
                    for eh in range(2):
                        pp = fill_psum()[:, 0:E // 2]
                        nc.tensor.matmul(pp, otab[Jb][:, ts(tt % 4, 128)],
                                         wpab_sb[:, ts(eh, E // 2)],
                                         start=True, stop=False)
                        nc.tensor.matmul(pp, ot2[Jb][:, ts(tt % 4, 128)],
                                         wp2x[:, ts(eh, E // 2)],
                                         start=False, stop=True)
                        nc.vector.tensor_copy(yt[:, ts(eh, E // 2)], pp)
                    nc.sync.dma_start(out=y[ts(tt, 128), :], in_=yt)

                # ------------- phase 2 (attention) emission -------------
                # filler units: [cost_ns, fn, gate, counted]; gate is a
                # tuple of finalize tags that must be emitted first (proj
                # units), or None. Gated-unready units are parked so they
                # never block the qkv force-drains behind them.
                fillers = []
                parked = []
                n_added = [0]
                n_drained = [0]
                marks = {}
                pending_fin = []
                fin_emitted = set()

                def add_fillers(units, mark=None, gate=None):
                    for cost, fn in units:
                        fillers.append([cost, fn, gate, False])
                    n_added[0] += len(units)
                    if mark is not None:
                        marks[mark] = n_added[0]

                def set_mark(mark, back=0):
                    marks[mark] = n_added[0] - back

                def _count(u):
                    if not u[3]:
                        u[3] = True
                        n_drained[0] += 1

                def _take():
                    while fillers:
                        u = fillers.pop(0)
                        _count(u)
                        if u[2] is None or all(t in fin_emitted
                                               for t in u[2]):
                            return u
                        parked.append(u)
                    return None

                def unpark():
                    ready = [u for u in parked
                             if all(t in fin_emitted for t in u[2])]
                    for u in ready:
                        parked.remove(u)
                    fillers[:0] = ready

                def drain_filler(budget):
                    while fillers and budget > 0:
                        u = _take()
                        if u is None:
                            return
                        u[1]()
                        budget -= u[0]

                def drain_all():
                    while fillers:
                        u = _take()
                        if u is not None:
                            u[1]()

                def drain_to(mark):
                    if mark not in marks:
                        return
                    while n_drained[0] < marks[mark] and fillers:
                        u = _take()
                        if u is not None:
                            u[1]()

                def c0_of(J, i):
                    r = i - 4 * J
                    return 0 if r < 0 else 128 * r

                def emit_av_group(h, J, g, otp, ex):
                    imax = 4 * J + 3
                    for u in range(GK):
                        i = GK * g + u
                        if i > imax:
                            break
                        r = i - 4 * J
                        c0 = c0_of(J, i)
                        if r >= 0:
                            # zero strictly-future keys in the diagonal
                            # 128x128 sub-block (tri[k,q] = k<=q)
                            nc.vector.tensor_mul(
                                ex[:, QB * u + c0: QB * u + c0 + 128],
                                ex[:, QB * u + c0: QB * u + c0 + 128],
                                tri_sb)
                        # O^T[d, q] (+ row 64 = denominator)
                        nc.tensor.matmul(
                            otp[:, c0:QB],
                            vtok[h][i // 4][:, (i % 4) * 65:(i % 4) * 65 + 65],
                            ex[:, QB * u + c0: QB * (u + 1)],
                            start=(i == 0), stop=(i == imax))

                def s_exp(spec, g):
                    hA, JA, hB, JB, qtX, ktX, dstA, dstB, mark, tag = spec
                    kA, kB = 4 * JA + 4, 4 * JB + 4
                    nA = (kA + GK - 1) // GK
                    nB = (kB + GK - 1) // GK
                    a, b = g < nA, g < nB
                    vA = min(GK, kA - GK * g) if a else 0
                    vB = min(GK, kB - GK * g) if b else 0
                    spA = sps.tile([128, GK * QB], F32, name="spA",
                                   tag="sp") if a else None
                    spB = sps.tile([128, GK * QB], F32, name="spB",
                                   tag="sp") if b else None
                    # interleave A/B per k-tile so row-packed pairs are
                    # adjacent in the PE queue
                    for u in range(GK):
                        if a and u < vA:
                            i = GK * g + u
                            c0 = c0_of(JA, i)
                            nc.tensor.matmul(
                                spA[:, QB * u + c0: QB * (u + 1)],
                                ktX[i // 4][0:64, ts(i % 4, 128)],
                                qtX[JA][0:64, c0:QB],
                                start=True, stop=True)
                        if b and u < vB:
                            i = GK * g + u
                            c0 = c0_of(JB, i)
                            nc.tensor.matmul(
                                spB[:, QB * u + c0: QB * (u + 1)],
                                ktX[i // 4][64:128, ts(i % 4, 128)],
                                qtX[JB][64:128, c0:QB],
                                start=True, stop=True)
                    exA = exB = None
                    if a:
                        exA = asb.tile([128, GK * QB], BF16, name="exA",
                                       tag="ex")
                        nc.scalar.activation(
                            exA[:, 0:vA * QB], spA[:, 0:vA * QB],
                            mybir.ActivationFunctionType.Exp)
                    if b:
                        exB = asb.tile([128, GK * QB], BF16, name="exB",
                                       tag="ex")
                        nc.scalar.activation(
                            exB[:, 0:vB * QB], spB[:, 0:vB * QB],
                            mybir.ActivationFunctionType.Exp)
                    return exA, exB

                head_ex = [None]

                def emit_pair(spec, next_head):
                    # Cross-pair software pipelining: this pair's first
                    # S/exp group was already emitted inside the previous
                    # pair's last group (head_ex); symmetrically, the next
                    # pair's head is emitted inside our second-to-last
                    # group. AV runs two groups behind S/exp so the
                    # previous pair's finalize broadcast (emitted at our
                    # g=1, after its recb is already computed) never
                    # blocks S matmuls in the PE queue, and its bct slot
                    # WAR resolves instantly.
                    hA, JA, hB, JB, qtX, ktX, dstA, dstB, mark, tag = spec
                    nA = (4 * JA + 4 + GK - 1) // GK
                    nB = (4 * JB + 4 + GK - 1) // GK
                    n = max(nA, nB)
                    if head_ex[0] is None:
                        drain_to(mark)
                        ex_q = [s_exp(spec, 0)]
                    else:
                        ex_q = [head_ex[0]]
                        head_ex[0] = None
                    otpA = otpB = None
                    for g in range(n + 1):
                        if g + 1 < n:
                            ex_q.append(s_exp(spec, g + 1))
                        elif g + 1 == n and next_head is not None:
                            next_head()
                        if g == 1:
                            for fz in pending_fin:
                                fz()
                            pending_fin.clear()
                            # allocate otp only after the previous pair's
                            # finalize reads are emitted (pool WAR tracking
                            # is snapshot-based)
                            otpA = ops.tile([65, QB], F32, name="otpA",
                                            tag="otp")
                            otpB = ops.tile([65, QB], F32, name="otpB",
                                            tag="otp")
                        if g >= 1:
                            drain_filler(1600)
                            # the V tiles this group's AV reads must be
                            # emitted before the AV matmuls (emission order
                            # IS the dependency order)
                            drain_to(f"v{min(GK * (g - 1) + GK - 1, 4 * max(JA, JB) + 3) // 4}")
                            exA, exB = ex_q.pop(0)
                            if exA is not None:
                                emit_av_group(hA, JA, g - 1, otpA, exA)
                            if exB is not None:
                                emit_av_group(hB, JB, g - 1, otpB, exB)

                    # ---- batched finalize, split in two ----
                    # ACT part now (right behind the last exps in the ACT
                    # queue): 1/den as exp(-ln(den)), both streams at once.
                    # denA lives at partition 0, denB at partition 32
                    # (partition bases must be 32-aligned); other rows are
                    # memset to 1.0 so Ln/Exp stay NaN-free.
                    den = nrm.tile([33, QB], F32, name="den", tag="den")
                    nc.vector.memset(den, 1.0)
                    nc.vector.tensor_copy(den[0:1, :], otpA[64:65, :])
                    nc.vector.tensor_copy(den[32:33, :], otpB[64:65, :])
                    lg = nrm.tile([33, QB], F32, name="lg", tag="lg")
                    nc.scalar.activation(lg, den,
                                         mybir.ActivationFunctionType.Ln)
                    recb = nrm.tile([33, QB], BF16, name="recb", tag="recb")
                    nc.scalar.activation(recb, lg,
                                         mybir.ActivationFunctionType.Exp,
                                         scale=-1.0)

                    # PE/DVE part deferred into the next pair (g=1), by
                    # which point recb is long done: the broadcast matmul
                    # and the normalizing multiplies
                    def finalize():
                        # broadcast: rows 0-63 = 1/denA, 64-127 = 1/denB
                        bct = sps.tile([128, GK * QB], F32, name="bct", tag="sp")
                        bcp = bct[:, 0:QB]
                        nc.tensor.matmul(bcp, bsel_sb, recb, start=True,
                                         stop=True)
                        bc = nrm.tile([128, QB], F32, name="bc", tag="bc")
                        nc.vector.tensor_copy(bc, bcp)
                        # stream B's O rows move to partitions 64-127 so the
                        # multiply stays base-aligned (DVE tensor_tensor
                        # cannot cross partition bases; copies can)
                        ob = nrm.tile([128, QB], F32, name="ob", tag="ob")
                        nc.vector.tensor_copy(ob[64:128, :], otpB[0:64, :])
                        nc.vector.tensor_mul(dstA[0:64, :], otpA[0:64, :],
                                             bc[0:64, :])
                        nc.vector.tensor_mul(dstB[64:128, :], ob[64:128, :],
                                             bc[64:128, :])
                        fin_emitted.add(tag)
                        unpark()

                    pending_fin.append(finalize)

                # ---------------- schedule ----------------
                def proj_units(Jb):
                    return [(900, lambda tt=tt: emit_proj(tt))
                            for tt in range(4 * Jb, 4 * Jb + 4)]

                def add_qkv(tb):
                    emit_x_dma(tb)
                    add_fillers(qkv_units(tb))
                    set_mark(f"qk{tb}", back=5)   # after f0,f1
                    set_mark(f"f2_{tb}", back=4)  # after f0,f1,f2
                    set_mark(f"v{tb}")            # after all V tiles

                plan = []

                def pair01(J):
                    plan.append(('p', (0, J, 1, J, qtab, ktab, otab[J],
                                       otab[J], f"qk{J}", f"p01_{J}")))

                def pair2(J):
                    plan.append(('p', (2, J, 2, J + 1, qt2, kt2, ot2[J],
                                       ot2[J + 1], f"f2_{J + 1}", f"p2_{J}")))

                def do(fn):
                    plan.append(('d', fn))

                # minimal prologue: only q/k of block 0, so the first
                # exps hit ACT a few us in; everything else is filler
                marks["qk0"] = 0
                for _, u in qk01_units(0):
                    u()
                add_fillers(rest_units(0))
                set_mark("f2_0", back=4)
                set_mark("v0")
                add_fillers(qk01_units(1), mark="qk1")
                add_fillers(rest_units(1))
                set_mark("f2_1", back=4)
                set_mark("v1")

                # pair order interleaves small-J and big-J pairs so the
                # filler supply stays roughly level; qkv lands just-in-time
                pair01(0)
                do(lambda: add_qkv(2))
                pair2(0)
                do(lambda: add_qkv(3))
                pair01(1)
                pair01(2)
                do(lambda: (add_fillers(proj_units(0), gate=("p01_0", "p2_0")),
                            add_fillers(proj_units(1), gate=("p01_1", "p2_0"))))
                pair01(3)
                do(lambda: add_qkv(4))
                pair2(2)
                do(lambda: add_qkv(5))
                pair01(4)
                do(lambda: (add_fillers(proj_units(2), gate=("p01_2", "p2_2")),
                            add_fillers(proj_units(3), gate=("p01_3", "p2_2"))))
                pair2(4)
                do(lambda: add_qkv(6))
                pair01(5)
                do(lambda: (add_fillers(proj_units(4), gate=("p01_4", "p2_4")),
                            add_fillers(proj_units(5), gate=("p01_5", "p2_4"))))
                do(lambda: add_qkv(7))
                pair01(6)
                pair2(6)
                do(lambda: add_fillers(proj_units(6), gate=("p01_6", "p2_6")))
                pair01(7)

                # ---- drive the plan with cross-pair head pipelining ----
                pidx = [i for i, (k, _) in enumerate(plan) if k == 'p']

                def make_head(pi, ni):
                    nspec = plan[ni][1]
                    dos = [v for (k, v) in plan[pi + 1:ni] if k == 'd']

                    def head():
                        for fn in dos:
                            fn()
                        drain_to(nspec[8])
                        head_ex[0] = s_exp(nspec, 0)
                    return head

                for k, v in plan[:pidx[0]]:
                    if k == 'd':
                        v()
                for j, pi in enumerate(pidx):
                    ni = pidx[j + 1] if j + 1 < len(pidx) else None
                    nh = make_head(pi, ni) if ni is not None else None
                    emit_pair(plan[pi][1], nh)
                for k, v in plan[pidx[-1] + 1:]:
                    if k == 'd':
                        v()
                for fz in pending_fin:
                    fz()
                pending_fin.clear()
                drain_all()

                # tail: last q-block's projection
                for tt in range(28, 32):
                    emit_proj(tt)

    _split_multi_waits(nc)
    return nc


def _get_nc(with_bias):
    if with_bias not in _nc:
        _nc[with_bias] = _build_program(with_bias)
    return _nc[with_bias]


def _bf16(a):
    return np.ascontiguousarray(a.astype(ml_dtypes.bfloat16))


def ts_(j):
    return slice(j * D, (j + 1) * D)


def kernel(x, W_attn, b_attn, W_proj, b_proj):
    x = np.asarray(x, dtype=np.float32)
    W_attn = np.asarray(W_attn, dtype=np.float32)
    b_attn = np.asarray(b_attn, dtype=np.float32)
    W_proj = np.asarray(W_proj, dtype=np.float32)
    b_proj = np.asarray(b_proj, dtype=np.float32)

    scale = 1.0 / np.sqrt(np.float32(D))
    with_bias = bool(np.any(b_attn != 0.0))
    EA = 832 if with_bias else 768

    # x^T per batch (optionally augmented with a ones row for the bias)
    xT_b = []
    for b in range(B):
        xa = np.zeros((EA, S), dtype=np.float32)
        xa[:E] = x[b].T
        if with_bias:
            xa[E] = 1.0
        xT_b.append(_bf16(xa))

    tri_np = _bf16(np.triu(np.ones((128, 128), dtype=np.float32)))
    bsel_np = np.zeros((33, 128), dtype=np.float32)
    bsel_np[0, 0:64] = 1.0
    bsel_np[32, 64:128] = 1.0
    bsel_np = _bf16(bsel_np)

    in_maps = []
    for c in range(NCORES):
        b = c // 4
        heads = [HPC * (c % 4) + j for j in range(HPC)]
        # wqk cols: [q_h0|q_h1|k_h0|k_h1|q_h2|k_h2]; q pre-scaled by 1/8
        wqk = np.zeros((EA, 2 * HPC * D), dtype=np.float32)
        wv = np.zeros((EA, HPC * D), dtype=np.float32)
        col_q = {0: 0, 1: 1, 2: 4}
        col_k = {0: 2, 1: 3, 2: 5}
        for j, h in enumerate(heads):
            wqk[:E, ts_(col_q[j])] = W_attn[:, h * D:(h + 1) * D] * scale
            wqk[:E, ts_(col_k[j])] = W_attn[:, E + h * D:E + (h + 1) * D]
            wv[:E, ts_(j)] = W_attn[:, 2 * E + h * D:2 * E + (h + 1) * D]
            if with_bias:
                wqk[E, ts_(col_q[j])] = b_attn[h * D:(h + 1) * D] * scale
                wqk[E, ts_(col_k[j])] = b_attn[E + h * D:E + (h + 1) * D]
                wv[E, ts_(j)] = b_attn[2 * E + h * D:2 * E + (h + 1) * D]
        wpab = np.concatenate(
            [W_proj[h * D:(h + 1) * D, :] for h in heads[:2]], axis=0)
        wp2 = W_proj[heads[2] * D:(heads[2] + 1) * D, :]
        in_maps.append({
            "xT": xT_b[b],
            "wqk": _bf16(wqk),
            "wv": _bf16(wv),
            "wpab": _bf16(wpab),
            "wp2": _bf16(wp2),
            "tri": tri_np,
            "bsel": bsel_np,
        })

    nc = _get_nc(with_bias)
    global LAST_EXEC_NS
    if TRACE:
        _install_ntff_hook()
        res = run_bass_kernel_spmd(nc, in_maps, core_ids=list(range(NCORES)),
                                   trace=True)
        LAST_EXEC_NS = res.exec_time_ns
    else:
        res = run_bass_kernel_spmd(nc, in_maps, core_ids=list(range(NCORES)))

    y = np.zeros((B, S, E), dtype=np.float32)
    for c in range(NCORES):
        y[c // 4] += res.results[c]["y"]
    y += b_proj
    return y


def _install_ntff_hook():
    """Register the axon NTFF profiling hook (dev/profiling only)."""
    import sys, types
    try:
        import antenv
        try:
            from antenv.axon_hooks import get_axon_ntff_profile_hook  # noqa
            return
        except ImportError:
            pass
        hooks_mod = types.ModuleType("antenv.axon_hooks")
        _hook = [None]
        hooks_mod.set_axon_ntff_profile_hook = lambda h: _hook.__setitem__(0, h)
        hooks_mod.get_axon_ntff_profile_hook = lambda: _hook[0]
        sys.modules["antenv.axon_hooks"] = hooks_mod
        antenv.axon_hooks = hooks_mod
        from trn_agent_boot.trn_boot import _ntff_profile_via_ctypes
        hooks_mod.set_axon_ntff_profile_hook(
            _ntff_profile_via_ctypes('/opt/axon/libaxon_pjrt.so'))
    except Exception:
        pass


# revision 56
# speedup vs baseline: 1.1146x; 1.1146x over previous
"""Causal self-attention (GPT-2 style) on 8 TRN2 NeuronCores.

Sharding: B=2 x H=12 -> 24 (batch, head) pairs; core c handles batch c//4
and heads [3*(c%4), 3*(c%4)+3). Each core computes QKV for its 3 heads,
causal attention (flash-style, scores^T layout), and a partial output
projection; the host sums the 4 per-batch partials and adds b_proj.

v2: ACT(exp)-bound design. The S^T matmuls run as K=64 row-packed pairs
(tile_position (0,0)/(64,0)) so two streams' score matmuls co-execute on
the PE; QKV and output-projection work is interleaved into the
attention slots as PE filler; softmax finalize is batched per stream
pair (one Ln+Exp on [2,512], one K=2 broadcast matmul via a [2,128]
selector). Per-head Q^T/K^T live in opposite partition halves
(h0/h1 packed; h2 duplicated) which also packs the projection
contraction to a full 128 rows. All persistent intermediates are
per-block tiles (single writer) so fused-phase dependency tracking
stays fine-grained.

Self-contained: builds the Bass program on first call, runs via
run_bass_kernel_spmd on cores 0-7.
"""
import numpy as np
import ml_dtypes

import concourse.bass as bass
import concourse.mybir as mybir
import concourse.tile as tile
from concourse.bass import ts
from concourse.vector_clock import ScopedClock
from concourse.bass_utils import run_bass_kernel_spmd

# ---------------------------------------------------------------------------
# Workaround for the container's walrus build, which rejects any instruction
# carrying more than ONE sync-wait command ("Too many sync wait commands").
# 1) patch the TileContext tail drain to funnel its wait-set through
#    single-wait NOPs on SP; 2) post-pass that moves excess on_wait entries
#    from any instruction onto single-wait NOPs inserted before it on the
#    same engine (engine stalls on the NOPs, then issues the instruction —
#    semantics preserved).
# ---------------------------------------------------------------------------
_WAIT_LIMIT = 1


def _patched_drain_and_barrier(self, tick_clock, wait_clock):
    nc = self.nc
    carrier = nc.sync.nop()
    wait_clock.add_sem_waits(carrier.ins, ScopedClock({None: tick_clock.global_clock}))
    si = carrier.ins.sync_info
    waits = list(si.on_wait) if si and si.on_wait else []
    if len(waits) > _WAIT_LIMIT:
        si.on_wait = waits[:_WAIT_LIMIT]
        for w in waits[_WAIT_LIMIT:]:
            n2 = nc.sync.nop()
            s2 = n2.ins.sync_info
            if s2 is None:
                n2.ins.sync_info = mybir.SyncInfo(on_wait=[w], on_update=[])
            else:
                s2.on_wait = [w]
    nc.sync.drain()
    nc.all_engine_barrier()
    popped = nc._tile_sem_poison_stack.pop()
    assert popped is self._sem_poison
    nc.clear_and_free_semaphores(list(self.sems.allocated().values()))
    nc.all_engine_barrier()


tile.TileContext._drain_and_barrier = _patched_drain_and_barrier


def _split_multi_waits(nc):
    n_inserted = 0
    for fn in nc.m.functions:
        for blk in fn.blocks:
            new_list = []
            changed = False
            for inst in blk.instructions:
                si = getattr(inst, "sync_info", None)
                waits = list(si.on_wait) if (si is not None and si.on_wait) else []
                if len(waits) > _WAIT_LIMIT:
                    extra = waits[: len(waits) - _WAIT_LIMIT]
                    keep = waits[len(waits) - _WAIT_LIMIT:]
                    for w in extra:
                        nop = mybir.InstNoOp(
                            name=f"wsplit-{n_inserted}",
                            sync_info=mybir.SyncInfo(on_wait=[w], on_update=[]),
                            bass_nofuse=True,
                            engine=inst.engine,
                        )
                        new_list.append(nop)
                        n_inserted += 1
                    si.on_wait = keep
                    changed = True
                new_list.append(inst)
            if changed:
                blk.instructions = new_list
    return n_inserted


# ---------------------------------------------------------------------------
# Problem constants (hardcoded per contract).
# ---------------------------------------------------------------------------
B, S, E, H = 2, 4096, 768, 12
D = 64           # head dim
HPC = 3          # heads per core
NCORES = 8
BF16 = mybir.dt.bfloat16
F32 = mybir.dt.float32
QB = 512         # q-block width
NQB = S // QB    # 8
NKT = S // 128   # 32 k-tiles

TRACE = False
LAST_EXEC_NS = None

_nc = {}


def _echunks(with_bias):
    # contraction chunks over the (augmented) feature dim
    ch = [(e * 128, 128) for e in range(6)]
    if with_bias:
        ch.append((768, 64))  # ones/bias row (+ zero padding)
    return ch


def _build_program(with_bias):
    nc = bass.Bass()
    EA = 832 if with_bias else 768
    ech = _echunks(with_bias)
    NE = len(ech)

    xT = nc.dram_tensor("xT", [EA, S], BF16, kind="ExternalInput")
    wqk = nc.dram_tensor("wqk", [EA, 2 * HPC * D], BF16, kind="ExternalInput")
    wv = nc.dram_tensor("wv", [EA, HPC * D], BF16, kind="ExternalInput")
    wpab = nc.dram_tensor("wpab", [128, E], BF16, kind="ExternalInput")
    wp2 = nc.dram_tensor("wp2", [D, E], BF16, kind="ExternalInput")
    tri = nc.dram_tensor("tri", [128, 128], BF16, kind="ExternalInput")
    bsel = nc.dram_tensor("bsel", [33, 128], BF16, kind="ExternalInput")
    y = nc.dram_tensor("y", [S, E], F32, kind="ExternalOutput")

    with tile.TileContext(nc) as tc:
        with (
            tc.tile_pool(name="wpool", bufs=1) as wpool,
            tc.tile_pool(name="per", bufs=1) as per,
            tc.tile_pool(name="xch", bufs=2) as xch,
            tc.tile_pool(name="sps", bufs=3, space="PSUM") as sps,
            tc.tile_pool(name="ops", bufs=2, space="PSUM") as ops,
            tc.tile_pool(name="asb", bufs=8) as asb,
            tc.tile_pool(name="nrm", bufs=2) as nrm,
            tc.tile_pool(name="ysb", bufs=2) as ysb,
        ):
            FQK = 2 * HPC * D
            xc_cur = {}

            def emit_x_dma(tb):
                xc = []
                for e, (r0, rn) in enumerate(ech):
                    t = xch.tile([rn, QB], BF16, name=f"xc{e}", tag=f"xc{e}")
                    nc.sync.dma_start(out=t, in_=xT[r0:r0 + rn, ts(tb, QB)])
                    xc.append(t)
                xc_cur[tb] = xc

            # --- weights to SBUF (critical-path first: wqk+x0 chunk
            # pairs interleaved so the first QK matmul can start after
            # two DMAs, not thirteen) ---
            bsel_sb = wpool.tile([33, 128], BF16, name="bsel_sb")
            nc.sync.dma_start(out=bsel_sb, in_=bsel[:, :])
            wqk_sb, wv_sb = [], []
            xc0 = []
            for e, (r0, rn) in enumerate(ech):
                t1 = wpool.tile([rn, FQK], BF16, name=f"wqk{e}")
                nc.sync.dma_start(out=t1, in_=wqk[r0:r0 + rn, :])
                wqk_sb.append(t1)
                t = xch.tile([rn, QB], BF16, name=f"xc{e}", tag=f"xc{e}")
                nc.sync.dma_start(out=t, in_=xT[r0:r0 + rn, 0:QB])
                xc0.append(t)
            xc_cur[0] = xc0
            emit_x_dma(1)
            for e, (r0, rn) in enumerate(ech):
                t2 = wpool.tile([rn, HPC * D], BF16, name=f"wv{e}")
                nc.sync.dma_start(out=t2, in_=wv[r0:r0 + rn, :])
                wv_sb.append(t2)
            # packed projection weights: wpab rows 0-63 = W_h0, 64-127 = W_h1
            wpab_sb = wpool.tile([128, E], BF16, name="wpab")
            nc.sync.dma_start(out=wpab_sb, in_=wpab[:, :])
            # h2 parity tiles: even q-blocks use rows 0-63 of ot2, odd 64-127
            wp2e = wpool.tile([128, E], BF16, name="wp2e")
            nc.sync.dma_start(out=wp2e[0:64, :], in_=wp2[:, :])
            nc.gpsimd.memset(wp2e[64:128, :], 0.0)
            wp2o = wpool.tile([128, E], BF16, name="wp2o")
            nc.gpsimd.memset(wp2o[0:64, :], 0.0)
            nc.sync.dma_start(out=wp2o[64:128, :], in_=wp2[:, :])
            tri_sb = wpool.tile([128, 128], BF16, name="tri_sb")
            nc.sync.dma_start(out=tri_sb, in_=tri[:, :])

            # --- persistent intermediates, per 512-token block ---
            # Feature-major Q^T/K^T: h0 in rows 0-63 + h1 in rows 64-127
            # (row-packed score matmuls); h2 duplicated into both halves.
            qtab = [per.tile([128, QB], BF16, name=f"qtab{j}") for j in range(NQB)]
            ktab = [per.tile([128, QB], BF16, name=f"ktab{j}") for j in range(NQB)]
            qt2 = [per.tile([128, QB], BF16, name=f"qt2_{j}") for j in range(NQB)]
            kt2 = [per.tile([128, QB], BF16, name=f"kt2_{j}") for j in range(NQB)]
            # vtok[h][tb]: token-major V with a ones column per k-tile:
            # cols [65s, 65s+64) = V rows of k-tile 4tb+s, col 65s+64 = 1.0
            vtok = [[per.tile([128, 65 * 4], BF16, name=f"vtok{h}_{j}")
                     for j in range(NQB)] for h in range(HPC)]
            for h in range(HPC):
                for j in range(NQB):
                    nc.vector.memset(vtok[h][j], 1.0)
            # normalized O^T per block: otab rows 0-63 = h0, 64-127 = h1;
            # ot2 rows 0-63 valid on even blocks, 64-127 on odd (other
            # half is garbage, masked by wp2e/wp2o zeros).
            otab = [per.tile([128, QB], BF16, name=f"otab{j}") for j in range(NQB)]
            ot2 = [per.tile([128, QB], BF16, name=f"ot2_{j}") for j in range(NQB)]
            for j in range(NQB):
                # the unused parity half must be zeros, not garbage:
                # 0 x NaN = NaN would poison the projection accumulation
                if j % 2 == 0:
                    nc.gpsimd.memset(ot2[j][64:128, :], 0.0)
                else:
                    nc.gpsimd.memset(ot2[j][0:64, :], 0.0)

            if True:
                # ACT spline-table preload: a throwaway exp during the
                # prologue pulls the ~2.7us ACT_TABLE_LOAD off phase 2's
                # critical path
                warm = nrm.tile([2, 128], F32, name="warm", tag="warm")
                nc.scalar.activation(warm, bsel_sb[0:2, :],
                                     mybir.ActivationFunctionType.Exp)
                def fill_psum():
                    # fillers borrow a score-pool slot (PSUM is fully
                    # subscribed: 3x2 sp banks + 2 otp banks)
                    return sps.tile([128, 2 * QB], F32, name="fp", tag="sp")

                # prime the PE's HAM clock gate during the initial DMA
                # wait: ~36 dummy matmuls on the tiny bsel tile give
                # ~4us of continuous PE activity so the real prologue
                # matmuls run at 2.4GHz instead of 1.2
                pump = fill_psum()
                for _ in range(36):
                    nc.tensor.matmul(pump[:, 0:128], bsel_sb, bsel_sb,
                                     start=True, stop=True)

                def emit_qk_ftile(tb, f):
                    # out rows = 128 cols of wqk f-tile; f0=[q0|q1]->qtab,
                    # f1=[k0|k1]->ktab, f2=[q2|k2]->qt2/kt2 duplicated
                    xc = xc_cur[tb]
                    ps = fill_psum()[:, 0:QB]
                    for e in range(NE):
                        nc.tensor.matmul(ps, wqk_sb[e][:, ts(f, 128)], xc[e],
                                         start=(e == 0), stop=(e == NE - 1))
                    if f == 0:
                        nc.vector.tensor_copy(qtab[tb], ps)
                    elif f == 1:
                        nc.vector.tensor_copy(ktab[tb], ps)
                    else:
                        nc.vector.tensor_copy(qt2[tb][0:64, :], ps[0:64, :])
                        nc.vector.tensor_copy(qt2[tb][64:128, :], ps[0:64, :])
                        nc.vector.tensor_copy(kt2[tb][0:64, :], ps[64:128, :])
                        nc.vector.tensor_copy(kt2[tb][64:128, :], ps[64:128, :])

                def emit_v_stile(tb, st):
                    xc = xc_cur[tb]
                    vp = fill_psum()[:, 0:HPC * D]
                    for e in range(NE):
                        nc.tensor.matmul(vp, xc[e][:, ts(st, 128)], wv_sb[e],
                                         start=(e == 0), stop=(e == NE - 1))
                    for h in range(HPC):
                        nc.vector.tensor_copy(
                            vtok[h][tb][:, st * 65: st * 65 + 64],
                            vp[:, ts(h, D)])
                    if st == 3:
                        del xc_cur[tb]

                def qkv_units(tb):
                    u = [(1300, lambda tb=tb, f=f: emit_qk_ftile(tb, f))
                         for f in range(3)]
                    u += [(700, lambda tb=tb, st=st: emit_v_stile(tb, st))
                          for st in range(4)]
                    return u

                def qk01_units(tb):
                    return [(1300, lambda tb=tb, f=f: emit_qk_ftile(tb, f))
                            for f in range(2)]

                def rest_units(tb):
                    u = [(1300, lambda tb=tb: emit_qk_ftile(tb, 2))]
                    u += [(700, lambda tb=tb, st=st: emit_v_stile(tb, st))
                          for st in range(4)]
                    return u

                def emit_proj(tt):
                    Jb = tt // 4
                    wp2x = wp2e if (Jb % 2 == 0) else wp2o
                    yt = ysb.tile([128, E], F32, name="yt", tag="yt")
                    for eh in range(2):
                        pp = fill_psum()[:, 0:E // 2]
                        nc.tensor.matmul(pp, otab[Jb][:, ts(tt % 4, 128)],
                                         wpab_sb[:, ts(eh, E // 2)],
                                         start=True, stop=False)
                        nc.tensor.matmul(pp, ot2[Jb][:, ts(tt % 4, 128)],
                                         wp2x[:, ts(eh, E // 2)],
                                         start=False, stop=True)
                        nc.vector.tensor_copy(yt[:, ts(eh, E // 2)], pp)
                    nc.sync.dma_start(out=y[ts(tt, 128), :], in_=yt)

                # ------------- phase 2 (attention) emission -------------
                # filler units: [cost_ns, fn, gate, counted]; gate is a
                # tuple of finalize tags that must be emitted first (proj
                # units), or None. Gated-unready units are parked so they
                # never block the qkv force-drains behind them.
                fillers = []
                parked = []
                n_added = [0]
                n_drained = [0]
                marks = {}
                pending_fin = []
                fin_emitted = set()

                def add_fillers(units, mark=None, gate=None):
                    for cost, fn in units:
                        fillers.append([cost, fn, gate, False])
                    n_added[0] += len(units)
                    if mark is not None:
                        marks[mark] = n_added[0]

                def set_mark(mark, back=0):
                    marks[mark] = n_added[0] - back

                def _count(u):
                    if not u[3]:
                        u[3] = True
                        n_drained[0] += 1

                def _take():
                    while fillers:
                        u = fillers.pop(0)
                        _count(u)
                        if u[2] is None or all(t in fin_emitted
                                               for t in u[2]):
                            return u
                        parked.append(u)
                    return None

                def unpark():
                    ready = [u for u in parked
                             if all(t in fin_emitted for t in u[2])]
                    for u in ready:
                        parked.remove(u)
                    fillers[:0] = ready

                def drain_filler(budget):
                    while fillers and budget > 0:
                        u = _take()
                        if u is None:
                            return
                        u[1]()
                        budget -= u[0]

                def drain_all():
                    while fillers:
                        u = _take()
                        if u is not None:
                            u[1]()

                def drain_to(mark):
                    if mark not in marks:
                        return
                    while n_drained[0] < marks[mark] and fillers:
                        u = _take()
                        if u is not None:
                            u[1]()

                def c0_of(J, i):
                    r = i - 4 * J
                    return 0 if r < 0 else 128 * r

                def emit_av_group(h, J, g, otp, ex):
                    imax = 4 * J + 3
                    for u in range(2):
                        i = 2 * g + u
                        r = i - 4 * J
                        c0 = c0_of(J, i)
                        if r >= 0:
                            # zero strictly-future keys in the diagonal
                            # 128x128 sub-block (tri[k,q] = k<=q)
                            nc.vector.tensor_mul(
                                ex[:, QB * u + c0: QB * u + c0 + 128],
                                ex[:, QB * u + c0: QB * u + c0 + 128],
                                tri_sb)
                        # O^T[d, q] (+ row 64 = denominator)
                        nc.tensor.matmul(
                            otp[:, c0:QB],
                            vtok[h][i // 4][:, (i % 4) * 65:(i % 4) * 65 + 65],
                            ex[:, QB * u + c0: QB * (u + 1)],
                            start=(i == 0), stop=(i == imax))

                def s_exp(spec, g):
                    hA, JA, hB, JB, qtX, ktX, dstA, dstB, mark, tag = spec
                    nA, nB = 2 * JA + 2, 2 * JB + 2
                    a, b = g < nA, g < nB
                    spA = sps.tile([128, 2 * QB], F32, name="spA",
                                   tag="sp") if a else None
                    spB = sps.tile([128, 2 * QB], F32, name="spB",
                                   tag="sp") if b else None
                    # interleave A/B per k-tile so row-packed pairs are
                    # adjacent in the PE queue
                    for u in range(2):
                        if a:
                            i = 2 * g + u
                            c0 = c0_of(JA, i)
                            nc.tensor.matmul(
                                spA[:, QB * u + c0: QB * (u + 1)],
                                ktX[i // 4][0:64, ts(i % 4, 128)],
                                qtX[JA][0:64, c0:QB],
                                start=True, stop=True)
                        if b:
                            i = 2 * g + u
                            c0 = c0_of(JB, i)
                            nc.tensor.matmul(
                                spB[:, QB * u + c0: QB * (u + 1)],
                                ktX[i // 4][64:128, ts(i % 4, 128)],
                                qtX[JB][64:128, c0:QB],
                                start=True, stop=True)
                    exA = exB = None
                    # start each exp at the first tile's causal offset: the
                    # masked prefix cols of diagonal groups are never read
                    # by AV, so skipping them is free ACT time
                    if a:
                        stA = c0_of(JA, 2 * g)
                        exA = asb.tile([128, 2 * QB], BF16, name="exA",
                                       tag="ex")
                        nc.scalar.activation(
                            exA[:, stA:], spA[:, stA:],
                            mybir.ActivationFunctionType.Exp)
                    if b:
                        stB = c0_of(JB, 2 * g)
                        exB = asb.tile([128, 2 * QB], BF16, name="exB",
                                       tag="ex")
                        nc.scalar.activation(
                            exB[:, stB:], spB[:, stB:],
                            mybir.ActivationFunctionType.Exp)
                    return exA, exB

                head_ex = [None]

                def emit_pair(spec, next_head):
                    # Cross-pair software pipelining: this pair's first
                    # S/exp group was already emitted inside the previous
                    # pair's last group (head_ex); symmetrically, the next
                    # pair's head is emitted inside our second-to-last
                    # group. AV runs two groups behind S/exp so the
                    # previous pair's finalize broadcast (emitted at our
                    # g=1, after its recb is already computed) never
                    # blocks S matmuls in the PE queue, and its bct slot
                    # WAR resolves instantly.
                    hA, JA, hB, JB, qtX, ktX, dstA, dstB, mark, tag = spec
                    nA, nB = 2 * JA + 2, 2 * JB + 2
                    n = max(nA, nB)
                    if head_ex[0] is None:
                        drain_to(mark)
                        ex_q = [s_exp(spec, 0)]
                    else:
                        ex_q = [head_ex[0]]
                        head_ex[0] = None
                    otpA = otpB = None
                    for g in range(n + 1):
                        if g + 1 < n:
                            ex_q.append(s_exp(spec, g + 1))
                        elif g + 1 == n and next_head is not None:
                            next_head()
                        if g == 1:
                            for fz in pending_fin:
                                fz()
                            pending_fin.clear()
                            # allocate otp only after the previous pair's
                            # finalize reads are emitted (pool WAR tracking
                            # is snapshot-based)
                            otpA = ops.tile([65, QB], F32, name="otpA",
                                            tag="otp")
                            otpB = ops.tile([65, QB], F32, name="otpB",
                                            tag="otp")
                        if g >= 1:
                            drain_filler(1600)
                            # the V tiles this group's AV reads must be
                            # emitted before the AV matmuls (emission order
                            # IS the dependency order)
                            drain_to(f"v{(2 * (g - 1) + 1) // 4}")
                            exA, exB = ex_q.pop(0)
                            if exA is not None:
                                emit_av_group(hA, JA, g - 1, otpA, exA)
                            if exB is not None:
                                emit_av_group(hB, JB, g - 1, otpB, exB)

                    # ---- batched finalize, split in two ----
                    # ACT part now (right behind the last exps in the ACT
                    # queue): 1/den as exp(-ln(den)), both streams at once.
                    # denA lives at partition 0, denB at partition 32
                    # (partition bases must be 32-aligned); other rows are
                    # memset to 1.0 so Ln/Exp stay NaN-free.
                    den = nrm.tile([33, QB], F32, name="den", tag="den")
                    nc.vector.memset(den, 1.0)
                    nc.vector.tensor_copy(den[0:1, :], otpA[64:65, :])
                    nc.vector.tensor_copy(den[32:33, :], otpB[64:65, :])
                    lg = nrm.tile([33, QB], F32, name="lg", tag="lg")
                    nc.scalar.activation(lg, den,
                                         mybir.ActivationFunctionType.Ln)
                    recb = nrm.tile([33, QB], BF16, name="recb", tag="recb")
                    nc.scalar.activation(recb, lg,
                                         mybir.ActivationFunctionType.Exp,
                                         scale=-1.0)

                    # PE/DVE part deferred into the next pair (g=1), by
                    # which point recb is long done: the broadcast matmul
                    # and the normalizing multiplies
                    def finalize():
                        # broadcast: rows 0-63 = 1/denA, 64-127 = 1/denB
                        bct = sps.tile([128, 2 * QB], F32, name="bct", tag="sp")
                        bcp = bct[:, 0:QB]
                        nc.tensor.matmul(bcp, bsel_sb, recb, start=True,
                                         stop=True)
                        bc = nrm.tile([128, QB], F32, name="bc", tag="bc")
                        nc.vector.tensor_copy(bc, bcp)
                        # stream B's O rows move to partitions 64-127 so the
                        # multiply stays base-aligned (DVE tensor_tensor
                        # cannot cross partition bases; copies can)
                        ob = nrm.tile([128, QB], F32, name="ob", tag="ob")
                        nc.vector.tensor_copy(ob[64:128, :], otpB[0:64, :])
                        nc.vector.tensor_mul(dstA[0:64, :], otpA[0:64, :],
                                             bc[0:64, :])
                        nc.vector.tensor_mul(dstB[64:128, :], ob[64:128, :],
                                             bc[64:128, :])
                        fin_emitted.add(tag)
                        unpark()

                    pending_fin.append(finalize)

                # ---------------- schedule ----------------
                def proj_units(Jb):
                    return [(900, lambda tt=tt: emit_proj(tt))
                            for tt in range(4 * Jb, 4 * Jb + 4)]

                def add_qkv(tb):
                    emit_x_dma(tb)
                    add_fillers(qkv_units(tb))
                    set_mark(f"qk{tb}", back=5)   # after f0,f1
                    set_mark(f"f2_{tb}", back=4)  # after f0,f1,f2
                    set_mark(f"v{tb}")            # after all V tiles

                plan = []

                def pair01(J):
                    plan.append(('p', (0, J, 1, J, qtab, ktab, otab[J],
                                       otab[J], f"qk{J}", f"p01_{J}")))

                def pair2(J):
                    plan.append(('p', (2, J, 2, J + 1, qt2, kt2, ot2[J],
                                       ot2[J + 1], f"f2_{J + 1}", f"p2_{J}")))

                def do(fn):
                    plan.append(('d', fn))

                # minimal prologue: only q/k of block 0, so the first
                # exps hit ACT a few us in; everything else is filler
                marks["qk0"] = 0
                for _, u in qk01_units(0):
                    u()
                add_fillers(rest_units(0))
                set_mark("f2_0", back=4)
                set_mark("v0")
                add_fillers(qk01_units(1), mark="qk1")
                add_fillers(rest_units(1))
                set_mark("f2_1", back=4)
                set_mark("v1")

                # pair order interleaves small-J and big-J pairs so the
                # filler supply stays roughly level; qkv lands just-in-time
                pair01(0)
                do(lambda: add_qkv(2))
                pair2(0)
                do(lambda: add_qkv(3))
                pair01(1)
                pair01(2)
                do(lambda: (add_fillers(proj_units(0), gate=("p01_0", "p2_0")),
                            add_fillers(proj_units(1), gate=("p01_1", "p2_0"))))
                pair01(3)
                do(lambda: add_qkv(4))
                pair2(2)
                do(lambda: add_qkv(5))
                pair01(4)
                do(lambda: (add_fillers(proj_units(2), gate=("p01_2", "p2_2")),
                            add_fillers(proj_units(3), gate=("p01_3", "p2_2"))))
                pair2(4)
                do(lambda: add_qkv(6))
                pair01(5)
                do(lambda: (add_fillers(proj_units(4), gate=("p01_4", "p2_4")),
                            add_fillers(proj_units(5), gate=("p01_5", "p2_4"))))
                do(lambda: add_qkv(7))
                pair01(6)
                pair2(6)
                do(lambda: add_fillers(proj_units(6), gate=("p01_6", "p2_6")))
                pair01(7)

                # ---- drive the plan with cross-pair head pipelining ----
                pidx = [i for i, (k, _) in enumerate(plan) if k == 'p']

                def make_head(pi, ni):
                    nspec = plan[ni][1]
                    dos = [v for (k, v) in plan[pi + 1:ni] if k == 'd']

                    def head():
                        for fn in dos:
                            fn()
                        drain_to(nspec[8])
                        head_ex[0] = s_exp(nspec, 0)
                    return head

                for k, v in plan[:pidx[0]]:
                    if k == 'd':
                        v()
                for j, pi in enumerate(pidx):
                    ni = pidx[j + 1] if j + 1 < len(pidx) else None
                    nh = make_head(pi, ni) if ni is not None else None
                    emit_pair(plan[pi][1], nh)
                for k, v in plan[pidx[-1] + 1:]:
                    if k == 'd':
                        v()
                for fz in pending_fin:
                    fz()
                pending_fin.clear()
                drain_all()

                # tail: last q-block's projection
                for tt in range(28, 32):
                    emit_proj(tt)

    _split_multi_waits(nc)
    return nc


def _get_nc(with_bias):
    if with_bias not in _nc:
        _nc[with_bias] = _build_program(with_bias)
    return _nc[with_bias]


def _bf16(a):
    return np.ascontiguousarray(a.astype(ml_dtypes.bfloat16))


def ts_(j):
    return slice(j * D, (j + 1) * D)


def kernel(x, W_attn, b_attn, W_proj, b_proj):
    x = np.asarray(x, dtype=np.float32)
    W_attn = np.asarray(W_attn, dtype=np.float32)
    b_attn = np.asarray(b_attn, dtype=np.float32)
    W_proj = np.asarray(W_proj, dtype=np.float32)
    b_proj = np.asarray(b_proj, dtype=np.float32)

    scale = 1.0 / np.sqrt(np.float32(D))
    with_bias = bool(np.any(b_attn != 0.0))
    EA = 832 if with_bias else 768

    # x^T per batch (optionally augmented with a ones row for the bias)
    xT_b = []
    for b in range(B):
        xa = np.zeros((EA, S), dtype=np.float32)
        xa[:E] = x[b].T
        if with_bias:
            xa[E] = 1.0
        xT_b.append(_bf16(xa))

    tri_np = _bf16(np.triu(np.ones((128, 128), dtype=np.float32)))
    bsel_np = np.zeros((33, 128), dtype=np.float32)
    bsel_np[0, 0:64] = 1.0
    bsel_np[32, 64:128] = 1.0
    bsel_np = _bf16(bsel_np)

    in_maps = []
    for c in range(NCORES):
        b = c // 4
        heads = [HPC * (c % 4) + j for j in range(HPC)]
        # wqk cols: [q_h0|q_h1|k_h0|k_h1|q_h2|k_h2]; q pre-scaled by 1/8
        wqk = np.zeros((EA, 2 * HPC * D), dtype=np.float32)
        wv = np.zeros((EA, HPC * D), dtype=np.float32)
        col_q = {0: 0, 1: 1, 2: 4}
        col_k = {0: 2, 1: 3, 2: 5}
        for j, h in enumerate(heads):
            wqk[:E, ts_(col_q[j])] = W_attn[:, h * D:(h + 1) * D] * scale
            wqk[:E, ts_(col_k[j])] = W_attn[:, E + h * D:E + (h + 1) * D]
            wv[:E, ts_(j)] = W_attn[:, 2 * E + h * D:2 * E + (h + 1) * D]
            if with_bias:
                wqk[E, ts_(col_q[j])] = b_attn[h * D:(h + 1) * D] * scale
                wqk[E, ts_(col_k[j])] = b_attn[E + h * D:E + (h + 1) * D]
                wv[E, ts_(j)] = b_attn[2 * E + h * D:2 * E + (h + 1) * D]
        wpab = np.concatenate(
            [W_proj[h * D:(h + 1) * D, :] for h in heads[:2]], axis=0)
        wp2 = W_proj[heads[2] * D:(heads[2] + 1) * D, :]
        in_maps.append({
            "xT": xT_b[b],
            "wqk": _bf16(wqk),
            "wv": _bf16(wv),
            "wpab": _bf16(wpab),
            "wp2": _bf16(wp2),
            "tri": tri_np,
            "bsel": bsel_np,
        })

    nc = _get_nc(with_bias)
    global LAST_EXEC_NS
    if TRACE:
        _install_ntff_hook()
        res = run_bass_kernel_spmd(nc, in_maps, core_ids=list(range(NCORES)),
                                   trace=True)
        LAST_EXEC_NS = res.exec_time_ns
    else:
        res = run_bass_kernel_spmd(nc, in_maps, core_ids=list(range(NCORES)))

    y = np.zeros((B, S, E), dtype=np.float32)
    for c in range(NCORES):
        y[c // 4] += res.results[c]["y"]
    y += b_proj
    return y


def _install_ntff_hook():
    """Register the axon NTFF profiling hook (dev/profiling only)."""
    import sys, types
    try:
        import antenv
        try:
            from antenv.axon_hooks import get_axon_ntff_profile_hook  # noqa
            return
        except ImportError:
            pass
        hooks_mod = types.ModuleType("antenv.axon_hooks")
        _hook = [None]
        hooks_mod.set_axon_ntff_profile_hook = lambda h: _hook.__setitem__(0, h)
        hooks_mod.get_axon_ntff_profile_hook = lambda: _hook[0]
        sys.modules["antenv.axon_hooks"] = hooks_mod
        antenv.axon_hooks = hooks_mod
        from trn_agent_boot.trn_boot import _ntff_profile_via_ctypes
        hooks_mod.set_axon_ntff_profile_hook(
            _ntff_profile_via_ctypes('/opt/axon/libaxon_pjrt.so'))
    except Exception:
        pass


# revision 57
# speedup vs baseline: 1.1324x; 1.0160x over previous
"""Causal self-attention (GPT-2 style) on 8 TRN2 NeuronCores.

Sharding: B=2 x H=12 -> 24 (batch, head) pairs; core c handles batch c//4
and heads [3*(c%4), 3*(c%4)+3). Each core computes QKV for its 3 heads,
causal attention (flash-style, scores^T layout), and a partial output
projection; the host sums the 4 per-batch partials and adds b_proj.

v2: ACT(exp)-bound design. The S^T matmuls run as K=64 row-packed pairs
(tile_position (0,0)/(64,0)) so two streams' score matmuls co-execute on
the PE; QKV and output-projection work is interleaved into the
attention slots as PE filler; softmax finalize is batched per stream
pair (one Ln+Exp on [2,512], one K=2 broadcast matmul via a [2,128]
selector). Per-head Q^T/K^T live in opposite partition halves
(h0/h1 packed; h2 duplicated) which also packs the projection
contraction to a full 128 rows. All persistent intermediates are
per-block tiles (single writer) so fused-phase dependency tracking
stays fine-grained.

Self-contained: builds the Bass program on first call, runs via
run_bass_kernel_spmd on cores 0-7.
"""
import numpy as np
import ml_dtypes

import concourse.bass as bass
import concourse.mybir as mybir
import concourse.tile as tile
from concourse.bass import ts
from concourse.vector_clock import ScopedClock
from concourse.bass_utils import run_bass_kernel_spmd

# ---------------------------------------------------------------------------
# Workaround for the container's walrus build, which rejects any instruction
# carrying more than ONE sync-wait command ("Too many sync wait commands").
# 1) patch the TileContext tail drain to funnel its wait-set through
#    single-wait NOPs on SP; 2) post-pass that moves excess on_wait entries
#    from any instruction onto single-wait NOPs inserted before it on the
#    same engine (engine stalls on the NOPs, then issues the instruction —
#    semantics preserved).
# ---------------------------------------------------------------------------
_WAIT_LIMIT = 1


def _patched_drain_and_barrier(self, tick_clock, wait_clock):
    nc = self.nc
    carrier = nc.sync.nop()
    wait_clock.add_sem_waits(carrier.ins, ScopedClock({None: tick_clock.global_clock}))
    si = carrier.ins.sync_info
    waits = list(si.on_wait) if si and si.on_wait else []
    if len(waits) > _WAIT_LIMIT:
        si.on_wait = waits[:_WAIT_LIMIT]
        for w in waits[_WAIT_LIMIT:]:
            n2 = nc.sync.nop()
            s2 = n2.ins.sync_info
            if s2 is None:
                n2.ins.sync_info = mybir.SyncInfo(on_wait=[w], on_update=[])
            else:
                s2.on_wait = [w]
    nc.sync.drain()
    nc.all_engine_barrier()
    popped = nc._tile_sem_poison_stack.pop()
    assert popped is self._sem_poison
    nc.clear_and_free_semaphores(list(self.sems.allocated().values()))
    nc.all_engine_barrier()


tile.TileContext._drain_and_barrier = _patched_drain_and_barrier


def _split_multi_waits(nc):
    n_inserted = 0
    for fn in nc.m.functions:
        for blk in fn.blocks:
            new_list = []
            changed = False
            for inst in blk.instructions:
                si = getattr(inst, "sync_info", None)
                waits = list(si.on_wait) if (si is not None and si.on_wait) else []
                if len(waits) > _WAIT_LIMIT:
                    extra = waits[: len(waits) - _WAIT_LIMIT]
                    keep = waits[len(waits) - _WAIT_LIMIT:]
                    for w in extra:
                        nop = mybir.InstNoOp(
                            name=f"wsplit-{n_inserted}",
                            sync_info=mybir.SyncInfo(on_wait=[w], on_update=[]),
                            bass_nofuse=True,
                            engine=inst.engine,
                        )
                        new_list.append(nop)
                        n_inserted += 1
                    si.on_wait = keep
                    changed = True
                new_list.append(inst)
            if changed:
                blk.instructions = new_list
    return n_inserted


# ---------------------------------------------------------------------------
# Problem constants (hardcoded per contract).
# ---------------------------------------------------------------------------
B, S, E, H = 2, 4096, 768, 12
D = 64           # head dim
HPC = 3          # heads per core
NCORES = 8
BF16 = mybir.dt.bfloat16
F32 = mybir.dt.float32
QB = 512         # q-block width
NQB = S // QB    # 8
NKT = S // 128   # 32 k-tiles

TRACE = False
LAST_EXEC_NS = None

_nc = {}


def _echunks(with_bias):
    # contraction chunks over the (augmented) feature dim
    ch = [(e * 128, 128) for e in range(6)]
    if with_bias:
        ch.append((768, 64))  # ones/bias row (+ zero padding)
    return ch


def _build_program(with_bias):
    nc = bass.Bass()
    EA = 832 if with_bias else 768
    ech = _echunks(with_bias)
    NE = len(ech)

    xT = nc.dram_tensor("xT", [EA, S], BF16, kind="ExternalInput")
    wqk = nc.dram_tensor("wqk", [EA, 2 * HPC * D], BF16, kind="ExternalInput")
    wv = nc.dram_tensor("wv", [EA, HPC * D], BF16, kind="ExternalInput")
    wpab = nc.dram_tensor("wpab", [128, E], BF16, kind="ExternalInput")
    wp2 = nc.dram_tensor("wp2", [D, E], BF16, kind="ExternalInput")
    tri = nc.dram_tensor("tri", [128, 128], BF16, kind="ExternalInput")
    bsel = nc.dram_tensor("bsel", [33, 128], BF16, kind="ExternalInput")
    y = nc.dram_tensor("y", [S, E], F32, kind="ExternalOutput")

    with tile.TileContext(nc) as tc:
        with (
            tc.tile_pool(name="wpool", bufs=1) as wpool,
            tc.tile_pool(name="per", bufs=1) as per,
            tc.tile_pool(name="xch", bufs=2) as xch,
            tc.tile_pool(name="sps", bufs=3, space="PSUM") as sps,
            tc.tile_pool(name="ops", bufs=2, space="PSUM") as ops,
            tc.tile_pool(name="asb", bufs=8) as asb,
            tc.tile_pool(name="nrm", bufs=2) as nrm,
            tc.tile_pool(name="ysb", bufs=2) as ysb,
        ):
            FQK = 2 * HPC * D
            xc_cur = {}

            def emit_x_dma(tb):
                # one 3D-AP DMA for all six 128-row feature chunks:
                # sync-queue issue costs ~600ns per DMA, so batching the
                # per-block loads 6->1 matters more than transfer shape
                t = xch.tile([128, 6 * QB], BF16, name="xc6", tag="xc6")
                nc.sync.dma_start(
                    out=t,
                    in_=xT[0:768, ts(tb, QB)].rearrange("(e p) c -> p e c",
                                                        e=6))
                xc = [t[:, e * QB:(e + 1) * QB] for e in range(6)]
                if with_bias:
                    tb_t = xch.tile([64, QB], BF16, name="xcb", tag="xcb")
                    nc.sync.dma_start(out=tb_t, in_=xT[768:832, ts(tb, QB)])
                    xc.append(tb_t)
                xc_cur[tb] = xc

            # --- weights to SBUF (critical-path first: wqk, x0, x1) ---
            bsel_sb = wpool.tile([33, 128], BF16, name="bsel_sb")
            nc.sync.dma_start(out=bsel_sb, in_=bsel[:, :])
            wqk_all = wpool.tile([128, 6 * FQK], BF16, name="wqk_all")
            nc.sync.dma_start(
                out=wqk_all,
                in_=wqk[0:768, :].rearrange("(e p) c -> p e c", e=6))
            wqk_sb = [wqk_all[:, e * FQK:(e + 1) * FQK] for e in range(6)]
            emit_x_dma(0)
            emit_x_dma(1)
            wv_all = wpool.tile([128, 6 * HPC * D], BF16, name="wv_all")
            nc.sync.dma_start(
                out=wv_all,
                in_=wv[0:768, :].rearrange("(e p) c -> p e c", e=6))
            wv_sb = [wv_all[:, e * HPC * D:(e + 1) * HPC * D]
                     for e in range(6)]
            if with_bias:
                wqkb = wpool.tile([64, FQK], BF16, name="wqkb")
                nc.sync.dma_start(out=wqkb, in_=wqk[768:832, :])
                wqk_sb.append(wqkb)
                wvb = wpool.tile([64, HPC * D], BF16, name="wvb")
                nc.sync.dma_start(out=wvb, in_=wv[768:832, :])
                wv_sb.append(wvb)
            # packed projection weights: wpab rows 0-63 = W_h0, 64-127 = W_h1
            wpab_sb = wpool.tile([128, E], BF16, name="wpab")
            nc.sync.dma_start(out=wpab_sb, in_=wpab[:, :])
            # h2 parity tiles: even q-blocks use rows 0-63 of ot2, odd 64-127
            wp2e = wpool.tile([128, E], BF16, name="wp2e")
            nc.sync.dma_start(out=wp2e[0:64, :], in_=wp2[:, :])
            nc.gpsimd.memset(wp2e[64:128, :], 0.0)
            wp2o = wpool.tile([128, E], BF16, name="wp2o")
            nc.gpsimd.memset(wp2o[0:64, :], 0.0)
            nc.sync.dma_start(out=wp2o[64:128, :], in_=wp2[:, :])
            tri_sb = wpool.tile([128, 128], BF16, name="tri_sb")
            nc.sync.dma_start(out=tri_sb, in_=tri[:, :])

            # --- persistent intermediates, per 512-token block ---
            # Feature-major Q^T/K^T: h0 in rows 0-63 + h1 in rows 64-127
            # (row-packed score matmuls); h2 duplicated into both halves.
            qtab = [per.tile([128, QB], BF16, name=f"qtab{j}") for j in range(NQB)]
            ktab = [per.tile([128, QB], BF16, name=f"ktab{j}") for j in range(NQB)]
            qt2 = [per.tile([128, QB], BF16, name=f"qt2_{j}") for j in range(NQB)]
            kt2 = [per.tile([128, QB], BF16, name=f"kt2_{j}") for j in range(NQB)]
            # vtok[h][tb]: token-major V with a ones column per k-tile:
            # cols [65s, 65s+64) = V rows of k-tile 4tb+s, col 65s+64 = 1.0
            vtok = [[per.tile([128, 65 * 4], BF16, name=f"vtok{h}_{j}")
                     for j in range(NQB)] for h in range(HPC)]
            for h in range(HPC):
                for j in range(NQB):
                    nc.vector.memset(vtok[h][j], 1.0)
            # normalized O^T per block: otab rows 0-63 = h0, 64-127 = h1;
            # ot2 rows 0-63 valid on even blocks, 64-127 on odd (other
            # half is garbage, masked by wp2e/wp2o zeros).
            otab = [per.tile([128, QB], BF16, name=f"otab{j}") for j in range(NQB)]
            ot2 = [per.tile([128, QB], BF16, name=f"ot2_{j}") for j in range(NQB)]
            for j in range(NQB):
                # the unused parity half must be zeros, not garbage:
                # 0 x NaN = NaN would poison the projection accumulation
                if j % 2 == 0:
                    nc.gpsimd.memset(ot2[j][64:128, :], 0.0)
                else:
                    nc.gpsimd.memset(ot2[j][0:64, :], 0.0)

            if True:
                # ACT spline-table preload: a throwaway exp during the
                # prologue pulls the ~2.7us ACT_TABLE_LOAD off phase 2's
                # critical path
                warm = nrm.tile([2, 128], F32, name="warm", tag="warm")
                nc.scalar.activation(warm, bsel_sb[0:2, :],
                                     mybir.ActivationFunctionType.Exp)
                def fill_psum():
                    # fillers borrow a score-pool slot (PSUM is fully
                    # subscribed: 3x2 sp banks + 2 otp banks)
                    return sps.tile([128, 2 * QB], F32, name="fp", tag="sp")

                # prime the PE's HAM clock gate during the initial DMA
                # wait: ~36 dummy matmuls on the tiny bsel tile give
                # ~4us of continuous PE activity so the real prologue
                # matmuls run at 2.4GHz instead of 1.2
                pump = fill_psum()
                for _ in range(36):
                    nc.tensor.matmul(pump[:, 0:128], bsel_sb, bsel_sb,
                                     start=True, stop=True)

                def emit_qk_ftile(tb, f):
                    # out rows = 128 cols of wqk f-tile; f0=[q0|q1]->qtab,
                    # f1=[k0|k1]->ktab, f2=[q2|k2]->qt2/kt2 duplicated
                    xc = xc_cur[tb]
                    ps = fill_psum()[:, 0:QB]
                    for e in range(NE):
                        nc.tensor.matmul(ps, wqk_sb[e][:, ts(f, 128)], xc[e],
                                         start=(e == 0), stop=(e == NE - 1))
                    if f == 0:
                        nc.vector.tensor_copy(qtab[tb], ps)
                    elif f == 1:
                        nc.vector.tensor_copy(ktab[tb], ps)
                    else:
                        nc.vector.tensor_copy(qt2[tb][0:64, :], ps[0:64, :])
                        nc.vector.tensor_copy(qt2[tb][64:128, :], ps[0:64, :])
                        nc.vector.tensor_copy(kt2[tb][0:64, :], ps[64:128, :])
                        nc.vector.tensor_copy(kt2[tb][64:128, :], ps[64:128, :])

                def emit_v_stile(tb, st):
                    xc = xc_cur[tb]
                    vp = fill_psum()[:, 0:HPC * D]
                    for e in range(NE):
                        nc.tensor.matmul(vp, xc[e][:, ts(st, 128)], wv_sb[e],
                                         start=(e == 0), stop=(e == NE - 1))
                    for h in range(HPC):
                        nc.vector.tensor_copy(
                            vtok[h][tb][:, st * 65: st * 65 + 64],
                            vp[:, ts(h, D)])
                    if st == 3:
                        del xc_cur[tb]

                def qkv_units(tb):
                    u = [(1300, lambda tb=tb, f=f: emit_qk_ftile(tb, f))
                         for f in range(3)]
                    u += [(700, lambda tb=tb, st=st: emit_v_stile(tb, st))
                          for st in range(4)]
                    return u

                def qk01_units(tb):
                    return [(1300, lambda tb=tb, f=f: emit_qk_ftile(tb, f))
                            for f in range(2)]

                def rest_units(tb):
                    u = [(1300, lambda tb=tb: emit_qk_ftile(tb, 2))]
                    u += [(700, lambda tb=tb, st=st: emit_v_stile(tb, st))
                          for st in range(4)]
                    return u

                def emit_proj(tt):
                    Jb = tt // 4
                    wp2x = wp2e if (Jb % 2 == 0) else wp2o
                    yt = ysb.tile([128, E], F32, name="yt", tag="yt")
                    for eh in range(2):
                        pp = fill_psum()[:, 0:E // 2]
                        nc.tensor.matmul(pp, otab[Jb][:, ts(tt % 4, 128)],
                                         wpab_sb[:, ts(eh, E // 2)],
                                         start=True, stop=False)
                        nc.tensor.matmul(pp, ot2[Jb][:, ts(tt % 4, 128)],
                                         wp2x[:, ts(eh, E // 2)],
                                         start=False, stop=True)
                        nc.vector.tensor_copy(yt[:, ts(eh, E // 2)], pp)
                    nc.sync.dma_start(out=y[ts(tt, 128), :], in_=yt)

                # ------------- phase 2 (attention) emission -------------
                # filler units: [cost_ns, fn, gate, counted]; gate is a
                # tuple of finalize tags that must be emitted first (proj
                # units), or None. Gated-unready units are parked so they
                # never block the qkv force-drains behind them.
                fillers = []
                parked = []
                n_added = [0]
                n_drained = [0]
                marks = {}
                pending_fin = []
                fin_emitted = set()

                def add_fillers(units, mark=None, gate=None):
                    for cost, fn in units:
                        fillers.append([cost, fn, gate, False])
                    n_added[0] += len(units)
                    if mark is not None:
                        marks[mark] = n_added[0]

                def set_mark(mark, back=0):
                    marks[mark] = n_added[0] - back

                def _count(u):
                    if not u[3]:
                        u[3] = True
                        n_drained[0] += 1

                def _take():
                    while fillers:
                        u = fillers.pop(0)
                        _count(u)
                        if u[2] is None or all(t in fin_emitted
                                               for t in u[2]):
                            return u
                        parked.append(u)
                    return None

                def unpark():
                    ready = [u for u in parked
                             if all(t in fin_emitted for t in u[2])]
                    for u in ready:
                        parked.remove(u)
                    fillers[:0] = ready

                def drain_filler(budget):
                    while fillers and budget > 0:
                        u = _take()
                        if u is None:
                            return
                        u[1]()
                        budget -= u[0]

                def drain_all():
                    while fillers:
                        u = _take()
                        if u is not None:
                            u[1]()

                def drain_to(mark):
                    if mark not in marks:
                        return
                    while n_drained[0] < marks[mark] and fillers:
                        u = _take()
                        if u is not None:
                            u[1]()

                def c0_of(J, i):
                    r = i - 4 * J
                    return 0 if r < 0 else 128 * r

                def emit_av_group(h, J, g, otp, ex):
                    imax = 4 * J + 3
                    for u in range(2):
                        i = 2 * g + u
                        r = i - 4 * J
                        c0 = c0_of(J, i)
                        if r >= 0:
                            # zero strictly-future keys in the diagonal
                            # 128x128 sub-block (tri[k,q] = k<=q)
                            nc.vector.tensor_mul(
                                ex[:, QB * u + c0: QB * u + c0 + 128],
                                ex[:, QB * u + c0: QB * u + c0 + 128],
                                tri_sb)
                        # O^T[d, q] (+ row 64 = denominator)
                        nc.tensor.matmul(
                            otp[:, c0:QB],
                            vtok[h][i // 4][:, (i % 4) * 65:(i % 4) * 65 + 65],
                            ex[:, QB * u + c0: QB * (u + 1)],
                            start=(i == 0), stop=(i == imax))

                def s_exp(spec, g):
                    hA, JA, hB, JB, qtX, ktX, dstA, dstB, mark, tag = spec
                    nA, nB = 2 * JA + 2, 2 * JB + 2
                    a, b = g < nA, g < nB
                    spA = sps.tile([128, 2 * QB], F32, name="spA",
                                   tag="sp") if a else None
                    spB = sps.tile([128, 2 * QB], F32, name="spB",
                                   tag="sp") if b else None
                    # interleave A/B per k-tile so row-packed pairs are
                    # adjacent in the PE queue
                    for u in range(2):
                        if a:
                            i = 2 * g + u
                            c0 = c0_of(JA, i)
                            nc.tensor.matmul(
                                spA[:, QB * u + c0: QB * (u + 1)],
                                ktX[i // 4][0:64, ts(i % 4, 128)],
                                qtX[JA][0:64, c0:QB],
                                start=True, stop=True)
                        if b:
                            i = 2 * g + u
                            c0 = c0_of(JB, i)
                            nc.tensor.matmul(
                                spB[:, QB * u + c0: QB * (u + 1)],
                                ktX[i // 4][64:128, ts(i % 4, 128)],
                                qtX[JB][64:128, c0:QB],
                                start=True, stop=True)
                    exA = exB = None
                    # start each exp at the first tile's causal offset: the
                    # masked prefix cols of diagonal groups are never read
                    # by AV, so skipping them is free ACT time
                    if a:
                        stA = c0_of(JA, 2 * g)
                        exA = asb.tile([128, 2 * QB], BF16, name="exA",
                                       tag="ex")
                        nc.scalar.activation(
                            exA[:, stA:], spA[:, stA:],
                            mybir.ActivationFunctionType.Exp)
                    if b:
                        stB = c0_of(JB, 2 * g)
                        exB = asb.tile([128, 2 * QB], BF16, name="exB",
                                       tag="ex")
                        nc.scalar.activation(
                            exB[:, stB:], spB[:, stB:],
                            mybir.ActivationFunctionType.Exp)
                    return exA, exB

                head_ex = [None]

                def emit_pair(spec, next_head):
                    # Cross-pair software pipelining: this pair's first
                    # S/exp group was already emitted inside the previous
                    # pair's last group (head_ex); symmetrically, the next
                    # pair's head is emitted inside our second-to-last
                    # group. AV runs two groups behind S/exp so the
                    # previous pair's finalize broadcast (emitted at our
                    # g=1, after its recb is already computed) never
                    # blocks S matmuls in the PE queue, and its bct slot
                    # WAR resolves instantly.
                    hA, JA, hB, JB, qtX, ktX, dstA, dstB, mark, tag = spec
                    nA, nB = 2 * JA + 2, 2 * JB + 2
                    n = max(nA, nB)
                    if head_ex[0] is None:
                        drain_to(mark)
                        ex_q = [s_exp(spec, 0)]
                    else:
                        ex_q = [head_ex[0]]
                        head_ex[0] = None
                    otpA = otpB = None
                    for g in range(n + 1):
                        if g + 1 < n:
                            ex_q.append(s_exp(spec, g + 1))
                        elif g + 1 == n and next_head is not None:
                            next_head()
                        if g == 1:
                            for fz in pending_fin:
                                fz()
                            pending_fin.clear()
                            # allocate otp only after the previous pair's
                            # finalize reads are emitted (pool WAR tracking
                            # is snapshot-based)
                            otpA = ops.tile([65, QB], F32, name="otpA",
                                            tag="otp")
                            otpB = ops.tile([65, QB], F32, name="otpB",
                                            tag="otp")
                        if g >= 1:
                            drain_filler(1600)
                            # the V tiles this group's AV reads must be
                            # emitted before the AV matmuls (emission order
                            # IS the dependency order)
                            drain_to(f"v{(2 * (g - 1) + 1) // 4}")
                            exA, exB = ex_q.pop(0)
                            if exA is not None:
                                emit_av_group(hA, JA, g - 1, otpA, exA)
                            if exB is not None:
                                emit_av_group(hB, JB, g - 1, otpB, exB)

                    # ---- batched finalize, split in two ----
                    # ACT part now (right behind the last exps in the ACT
                    # queue): 1/den as exp(-ln(den)), both streams at once.
                    # denA lives at partition 0, denB at partition 32
                    # (partition bases must be 32-aligned); other rows are
                    # memset to 1.0 so Ln/Exp stay NaN-free.
                    den = nrm.tile([33, QB], F32, name="den", tag="den")
                    nc.vector.memset(den, 1.0)
                    nc.vector.tensor_copy(den[0:1, :], otpA[64:65, :])
                    nc.vector.tensor_copy(den[32:33, :], otpB[64:65, :])
                    lg = nrm.tile([33, QB], F32, name="lg", tag="lg")
                    nc.scalar.activation(lg, den,
                                         mybir.ActivationFunctionType.Ln)
                    recb = nrm.tile([33, QB], BF16, name="recb", tag="recb")
                    nc.scalar.activation(recb, lg,
                                         mybir.ActivationFunctionType.Exp,
                                         scale=-1.0)

                    # PE/DVE part deferred into the next pair (g=1), by
                    # which point recb is long done: the broadcast matmul
                    # and the normalizing multiplies
                    def finalize():
                        # broadcast: rows 0-63 = 1/denA, 64-127 = 1/denB
                        bct = sps.tile([128, 2 * QB], F32, name="bct", tag="sp")
                        bcp = bct[:, 0:QB]
                        nc.tensor.matmul(bcp, bsel_sb, recb, start=True,
                                         stop=True)
                        bc = nrm.tile([128, QB], F32, name="bc", tag="bc")
                        nc.vector.tensor_copy(bc, bcp)
                        # stream B's O rows move to partitions 64-127 so the
                        # multiply stays base-aligned (DVE tensor_tensor
                        # cannot cross partition bases; copies can)
                        ob = nrm.tile([128, QB], F32, name="ob", tag="ob")
                        nc.vector.tensor_copy(ob[64:128, :], otpB[0:64, :])
                        nc.vector.tensor_mul(dstA[0:64, :], otpA[0:64, :],
                                             bc[0:64, :])
                        nc.vector.tensor_mul(dstB[64:128, :], ob[64:128, :],
                                             bc[64:128, :])
                        fin_emitted.add(tag)
                        unpark()

                    pending_fin.append(finalize)

                # ---------------- schedule ----------------
                def proj_units(Jb):
                    return [(900, lambda tt=tt: emit_proj(tt))
                            for tt in range(4 * Jb, 4 * Jb + 4)]

                def add_qkv(tb):
                    emit_x_dma(tb)
                    add_fillers(qkv_units(tb))
                    set_mark(f"qk{tb}", back=5)   # after f0,f1
                    set_mark(f"f2_{tb}", back=4)  # after f0,f1,f2
                    set_mark(f"v{tb}")            # after all V tiles

                plan = []

                def pair01(J):
                    plan.append(('p', (0, J, 1, J, qtab, ktab, otab[J],
                                       otab[J], f"qk{J}", f"p01_{J}")))

                def pair2(J):
                    plan.append(('p', (2, J, 2, J + 1, qt2, kt2, ot2[J],
                                       ot2[J + 1], f"f2_{J + 1}", f"p2_{J}")))

                def do(fn):
                    plan.append(('d', fn))

                # minimal prologue: only q/k of block 0, so the first
                # exps hit ACT a few us in; everything else is filler
                marks["qk0"] = 0
                for _, u in qk01_units(0):
                    u()
                add_fillers(rest_units(0))
                set_mark("f2_0", back=4)
                set_mark("v0")
                add_fillers(qk01_units(1), mark="qk1")
                add_fillers(rest_units(1))
                set_mark("f2_1", back=4)
                set_mark("v1")

                # pair order interleaves small-J and big-J pairs so the
                # filler supply stays roughly level; qkv lands just-in-time
                pair01(0)
                do(lambda: add_qkv(2))
                pair2(0)
                do(lambda: add_qkv(3))
                pair01(1)
                pair01(2)
                do(lambda: (add_fillers(proj_units(0), gate=("p01_0", "p2_0")),
                            add_fillers(proj_units(1), gate=("p01_1", "p2_0"))))
                pair01(3)
                do(lambda: add_qkv(4))
                pair2(2)
                do(lambda: add_qkv(5))
                pair01(4)
                do(lambda: (add_fillers(proj_units(2), gate=("p01_2", "p2_2")),
                            add_fillers(proj_units(3), gate=("p01_3", "p2_2"))))
                pair2(4)
                do(lambda: add_qkv(6))
                pair01(5)
                do(lambda: (add_fillers(proj_units(4), gate=("p01_4", "p2_4")),
                            add_fillers(proj_units(5), gate=("p01_5", "p2_4"))))
                do(lambda: add_qkv(7))
                pair01(6)
                pair2(6)
                do(lambda: add_fillers(proj_units(6), gate=("p01_6", "p2_6")))
                pair01(7)

                # ---- drive the plan with cross-pair head pipelining ----
                pidx = [i for i, (k, _) in enumerate(plan) if k == 'p']

                def make_head(pi, ni):
                    nspec = plan[ni][1]
                    dos = [v for (k, v) in plan[pi + 1:ni] if k == 'd']

                    def head():
                        for fn in dos:
                            fn()
                        drain_to(nspec[8])
                        head_ex[0] = s_exp(nspec, 0)
                    return head

                for k, v in plan[:pidx[0]]:
                    if k == 'd':
                        v()
                for j, pi in enumerate(pidx):
                    ni = pidx[j + 1] if j + 1 < len(pidx) else None
                    nh = make_head(pi, ni) if ni is not None else None
                    emit_pair(plan[pi][1], nh)
                for k, v in plan[pidx[-1] + 1:]:
                    if k == 'd':
                        v()
                for fz in pending_fin:
                    fz()
                pending_fin.clear()
                drain_all()

                # tail: last q-block's projection
                for tt in range(28, 32):
                    emit_proj(tt)

    _split_multi_waits(nc)
    return nc


def _get_nc(with_bias):
    if with_bias not in _nc:
        _nc[with_bias] = _build_program(with_bias)
    return _nc[with_bias]


def _bf16(a):
    return np.ascontiguousarray(a.astype(ml_dtypes.bfloat16))


def ts_(j):
    return slice(j * D, (j + 1) * D)


def kernel(x, W_attn, b_attn, W_proj, b_proj):
    x = np.asarray(x, dtype=np.float32)
    W_attn = np.asarray(W_attn, dtype=np.float32)
    b_attn = np.asarray(b_attn, dtype=np.float32)
    W_proj = np.asarray(W_proj, dtype=np.float32)
    b_proj = np.asarray(b_proj, dtype=np.float32)

    scale = 1.0 / np.sqrt(np.float32(D))
    with_bias = bool(np.any(b_attn != 0.0))
    EA = 832 if with_bias else 768

    # x^T per batch (optionally augmented with a ones row for the bias)
    xT_b = []
    for b in range(B):
        xa = np.zeros((EA, S), dtype=np.float32)
        xa[:E] = x[b].T
        if with_bias:
            xa[E] = 1.0
        xT_b.append(_bf16(xa))

    tri_np = _bf16(np.triu(np.ones((128, 128), dtype=np.float32)))
    bsel_np = np.zeros((33, 128), dtype=np.float32)
    bsel_np[0, 0:64] = 1.0
    bsel_np[32, 64:128] = 1.0
    bsel_np = _bf16(bsel_np)

    in_maps = []
    for c in range(NCORES):
        b = c // 4
        heads = [HPC * (c % 4) + j for j in range(HPC)]
        # wqk cols: [q_h0|q_h1|k_h0|k_h1|q_h2|k_h2]; q pre-scaled by 1/8
        wqk = np.zeros((EA, 2 * HPC * D), dtype=np.float32)
        wv = np.zeros((EA, HPC * D), dtype=np.float32)
        col_q = {0: 0, 1: 1, 2: 4}
        col_k = {0: 2, 1: 3, 2: 5}
        for j, h in enumerate(heads):
            wqk[:E, ts_(col_q[j])] = W_attn[:, h * D:(h + 1) * D] * scale
            wqk[:E, ts_(col_k[j])] = W_attn[:, E + h * D:E + (h + 1) * D]
            wv[:E, ts_(j)] = W_attn[:, 2 * E + h * D:2 * E + (h + 1) * D]
            if with_bias:
                wqk[E, ts_(col_q[j])] = b_attn[h * D:(h + 1) * D] * scale
                wqk[E, ts_(col_k[j])] = b_attn[E + h * D:E + (h + 1) * D]
                wv[E, ts_(j)] = b_attn[2 * E + h * D:2 * E + (h + 1) * D]
        wpab = np.concatenate(
            [W_proj[h * D:(h + 1) * D, :] for h in heads[:2]], axis=0)
        wp2 = W_proj[heads[2] * D:(heads[2] + 1) * D, :]
        in_maps.append({
            "xT": xT_b[b],
            "wqk": _bf16(wqk),
            "wv": _bf16(wv),
            "wpab": _bf16(wpab),
            "wp2": _bf16(wp2),
            "tri": tri_np,
            "bsel": bsel_np,
        })

    nc = _get_nc(with_bias)
    global LAST_EXEC_NS
    if TRACE:
        _install_ntff_hook()
        res = run_bass_kernel_spmd(nc, in_maps, core_ids=list(range(NCORES)),
                                   trace=True)
        LAST_EXEC_NS = res.exec_time_ns
    else:
        res = run_bass_kernel_spmd(nc, in_maps, core_ids=list(range(NCORES)))

    y = np.zeros((B, S, E), dtype=np.float32)
    for c in range(NCORES):
        y[c // 4] += res.results[c]["y"]
    y += b_proj
    return y


def _install_ntff_hook():
    """Register the axon NTFF profiling hook (dev/profiling only)."""
    import sys, types
    try:
        import antenv
        try:
            from antenv.axon_hooks import get_axon_ntff_profile_hook  # noqa
            return
        except ImportError:
            pass
        hooks_mod = types.ModuleType("antenv.axon_hooks")
        _hook = [None]
        hooks_mod.set_axon_ntff_profile_hook = lambda h: _hook.__setitem__(0, h)
        hooks_mod.get_axon_ntff_profile_hook = lambda: _hook[0]
        sys.modules["antenv.axon_hooks"] = hooks_mod
        antenv.axon_hooks = hooks_mod
        from trn_agent_boot.trn_boot import _ntff_profile_via_ctypes
        hooks_mod.set_axon_ntff_profile_hook(
            _ntff_profile_via_ctypes('/opt/axon/libaxon_pjrt.so'))
    except Exception:
        pass


# revision 58
# speedup vs baseline: 1.1428x; 1.0091x over previous
"""Causal self-attention (GPT-2 style) on 8 TRN2 NeuronCores.

Sharding: B=2 x H=12 -> 24 (batch, head) pairs; core c handles batch c//4
and heads [3*(c%4), 3*(c%4)+3). Each core computes QKV for its 3 heads,
causal attention (flash-style, scores^T layout), and a partial output
projection; the host sums the 4 per-batch partials and adds b_proj.

v2: ACT(exp)-bound design. The S^T matmuls run as K=64 row-packed pairs
(tile_position (0,0)/(64,0)) so two streams' score matmuls co-execute on
the PE; QKV and output-projection work is interleaved into the
attention slots as PE filler; softmax finalize is batched per stream
pair (one Ln+Exp on [2,512], one K=2 broadcast matmul via a [2,128]
selector). Per-head Q^T/K^T live in opposite partition halves
(h0/h1 packed; h2 duplicated) which also packs the projection
contraction to a full 128 rows. All persistent intermediates are
per-block tiles (single writer) so fused-phase dependency tracking
stays fine-grained.

Self-contained: builds the Bass program on first call, runs via
run_bass_kernel_spmd on cores 0-7.
"""
import numpy as np
import ml_dtypes

import concourse.bass as bass
import concourse.mybir as mybir
import concourse.tile as tile
from concourse.bass import ts
from concourse.vector_clock import ScopedClock
from concourse.bass_utils import run_bass_kernel_spmd

# ---------------------------------------------------------------------------
# Workaround for the container's walrus build, which rejects any instruction
# carrying more than ONE sync-wait command ("Too many sync wait commands").
# 1) patch the TileContext tail drain to funnel its wait-set through
#    single-wait NOPs on SP; 2) post-pass that moves excess on_wait entries
#    from any instruction onto single-wait NOPs inserted before it on the
#    same engine (engine stalls on the NOPs, then issues the instruction —
#    semantics preserved).
# ---------------------------------------------------------------------------
_WAIT_LIMIT = 1


def _patched_drain_and_barrier(self, tick_clock, wait_clock):
    nc = self.nc
    carrier = nc.sync.nop()
    wait_clock.add_sem_waits(carrier.ins, ScopedClock({None: tick_clock.global_clock}))
    si = carrier.ins.sync_info
    waits = list(si.on_wait) if si and si.on_wait else []
    if len(waits) > _WAIT_LIMIT:
        si.on_wait = waits[:_WAIT_LIMIT]
        for w in waits[_WAIT_LIMIT:]:
            n2 = nc.sync.nop()
            s2 = n2.ins.sync_info
            if s2 is None:
                n2.ins.sync_info = mybir.SyncInfo(on_wait=[w], on_update=[])
            else:
                s2.on_wait = [w]
    nc.sync.drain()
    nc.all_engine_barrier()
    popped = nc._tile_sem_poison_stack.pop()
    assert popped is self._sem_poison
    nc.clear_and_free_semaphores(list(self.sems.allocated().values()))
    nc.all_engine_barrier()


tile.TileContext._drain_and_barrier = _patched_drain_and_barrier


def _split_multi_waits(nc):
    n_inserted = 0
    for fn in nc.m.functions:
        for blk in fn.blocks:
            new_list = []
            changed = False
            for inst in blk.instructions:
                si = getattr(inst, "sync_info", None)
                waits = list(si.on_wait) if (si is not None and si.on_wait) else []
                if len(waits) > _WAIT_LIMIT:
                    extra = waits[: len(waits) - _WAIT_LIMIT]
                    keep = waits[len(waits) - _WAIT_LIMIT:]
                    for w in extra:
                        nop = mybir.InstNoOp(
                            name=f"wsplit-{n_inserted}",
                            sync_info=mybir.SyncInfo(on_wait=[w], on_update=[]),
                            bass_nofuse=True,
                            engine=inst.engine,
                        )
                        new_list.append(nop)
                        n_inserted += 1
                    si.on_wait = keep
                    changed = True
                new_list.append(inst)
            if changed:
                blk.instructions = new_list
    return n_inserted


# ---------------------------------------------------------------------------
# Problem constants (hardcoded per contract).
# ---------------------------------------------------------------------------
B, S, E, H = 2, 4096, 768, 12
D = 64           # head dim
HPC = 3          # heads per core
NCORES = 8
BF16 = mybir.dt.bfloat16
F32 = mybir.dt.float32
QB = 512         # q-block width
NQB = S // QB    # 8
NKT = S // 128   # 32 k-tiles

TRACE = False
LAST_EXEC_NS = None

_nc = {}


def _echunks(with_bias):
    # contraction chunks over the (augmented) feature dim
    ch = [(e * 128, 128) for e in range(6)]
    if with_bias:
        ch.append((768, 64))  # ones/bias row (+ zero padding)
    return ch


def _build_program(with_bias):
    nc = bass.Bass()
    EA = 832 if with_bias else 768
    ech = _echunks(with_bias)
    NE = len(ech)

    xT = nc.dram_tensor("xT", [EA, S], BF16, kind="ExternalInput")
    wqk = nc.dram_tensor("wqk", [EA, 2 * HPC * D], BF16, kind="ExternalInput")
    wv = nc.dram_tensor("wv", [EA, HPC * D], BF16, kind="ExternalInput")
    wpab = nc.dram_tensor("wpab", [128, E], BF16, kind="ExternalInput")
    wp2 = nc.dram_tensor("wp2", [D, E], BF16, kind="ExternalInput")
    tri = nc.dram_tensor("tri", [128, 128], BF16, kind="ExternalInput")
    bsel = nc.dram_tensor("bsel", [33, 128], BF16, kind="ExternalInput")
    y = nc.dram_tensor("y", [S, E], F32, kind="ExternalOutput")

    with tile.TileContext(nc) as tc:
        with (
            tc.tile_pool(name="wpool", bufs=1) as wpool,
            tc.tile_pool(name="per", bufs=1) as per,
            tc.tile_pool(name="xch", bufs=2) as xch,
            tc.tile_pool(name="sps", bufs=3, space="PSUM") as sps,
            tc.tile_pool(name="ops", bufs=2, space="PSUM") as ops,
            tc.tile_pool(name="asb", bufs=8) as asb,
            tc.tile_pool(name="nrm", bufs=2) as nrm,
            tc.tile_pool(name="ysb", bufs=2) as ysb,
        ):
            FQK = 2 * HPC * D
            xc_cur = {}

            def emit_x_dma(tb):
                # one 3D-AP DMA for all six 128-row feature chunks:
                # sync-queue issue costs ~600ns per DMA, so batching the
                # per-block loads 6->1 matters more than transfer shape
                t = xch.tile([128, 6 * QB], BF16, name="xc6", tag="xc6")
                nc.sync.dma_start(
                    out=t,
                    in_=xT[0:768, ts(tb, QB)].rearrange("(e p) c -> p e c",
                                                        e=6))
                xc = [t[:, e * QB:(e + 1) * QB] for e in range(6)]
                if with_bias:
                    tb_t = xch.tile([64, QB], BF16, name="xcb", tag="xcb")
                    nc.sync.dma_start(out=tb_t, in_=xT[768:832, ts(tb, QB)])
                    xc.append(tb_t)
                xc_cur[tb] = xc

            # --- weights to SBUF (critical-path first: wqk, x0, x1) ---
            bsel_sb = wpool.tile([33, 128], BF16, name="bsel_sb")
            nc.sync.dma_start(out=bsel_sb, in_=bsel[:, :])
            wqk_all = wpool.tile([128, 6 * FQK], BF16, name="wqk_all")
            nc.sync.dma_start(
                out=wqk_all,
                in_=wqk[0:768, :].rearrange("(e p) c -> p e c", e=6))
            wqk_sb = [wqk_all[:, e * FQK:(e + 1) * FQK] for e in range(6)]
            emit_x_dma(0)
            emit_x_dma(1)
            wv_all = wpool.tile([128, 6 * HPC * D], BF16, name="wv_all")
            nc.sync.dma_start(
                out=wv_all,
                in_=wv[0:768, :].rearrange("(e p) c -> p e c", e=6))
            wv_sb = [wv_all[:, e * HPC * D:(e + 1) * HPC * D]
                     for e in range(6)]
            if with_bias:
                wqkb = wpool.tile([64, FQK], BF16, name="wqkb")
                nc.sync.dma_start(out=wqkb, in_=wqk[768:832, :])
                wqk_sb.append(wqkb)
                wvb = wpool.tile([64, HPC * D], BF16, name="wvb")
                nc.sync.dma_start(out=wvb, in_=wv[768:832, :])
                wv_sb.append(wvb)
            # packed projection weights: wpab rows 0-63 = W_h0, 64-127 = W_h1
            wpab_sb = wpool.tile([128, E], BF16, name="wpab")
            nc.sync.dma_start(out=wpab_sb, in_=wpab[:, :])
            # h2 parity tiles: even q-blocks use rows 0-63 of ot2, odd 64-127
            wp2e = wpool.tile([128, E], BF16, name="wp2e")
            nc.sync.dma_start(out=wp2e[0:64, :], in_=wp2[:, :])
            nc.gpsimd.memset(wp2e[64:128, :], 0.0)
            wp2o = wpool.tile([128, E], BF16, name="wp2o")
            nc.gpsimd.memset(wp2o[0:64, :], 0.0)
            nc.sync.dma_start(out=wp2o[64:128, :], in_=wp2[:, :])
            tri_sb = wpool.tile([128, 128], BF16, name="tri_sb")
            nc.sync.dma_start(out=tri_sb, in_=tri[:, :])

            # --- persistent intermediates, per 512-token block ---
            # Feature-major Q^T/K^T: h0 in rows 0-63 + h1 in rows 64-127
            # (row-packed score matmuls); h2 duplicated into both halves.
            qtab = [per.tile([128, QB], BF16, name=f"qtab{j}") for j in range(NQB)]
            ktab = [per.tile([128, QB], BF16, name=f"ktab{j}") for j in range(NQB)]
            qt2 = [per.tile([128, QB], BF16, name=f"qt2_{j}") for j in range(NQB)]
            kt2 = [per.tile([128, QB], BF16, name=f"kt2_{j}") for j in range(NQB)]
            # vtok[h][tb]: token-major V with a ones column per k-tile:
            # cols [65s, 65s+64) = V rows of k-tile 4tb+s, col 65s+64 = 1.0
            vtok = [[per.tile([128, 65 * 4], BF16, name=f"vtok{h}_{j}")
                     for j in range(NQB)] for h in range(HPC)]
            for h in range(HPC):
                for j in range(NQB):
                    nc.vector.memset(vtok[h][j], 1.0)
            # normalized O^T per block: otab rows 0-63 = h0, 64-127 = h1;
            # ot2 rows 0-63 valid on even blocks, 64-127 on odd (other
            # half is garbage, masked by wp2e/wp2o zeros).
            otab = [per.tile([128, QB], BF16, name=f"otab{j}") for j in range(NQB)]
            ot2 = [per.tile([128, QB], BF16, name=f"ot2_{j}") for j in range(NQB)]
            for j in range(NQB):
                # the unused parity half must be zeros, not garbage:
                # 0 x NaN = NaN would poison the projection accumulation
                if j % 2 == 0:
                    nc.gpsimd.memset(ot2[j][64:128, :], 0.0)
                else:
                    nc.gpsimd.memset(ot2[j][0:64, :], 0.0)

            if True:
                # ACT spline-table preload: a throwaway exp during the
                # prologue pulls the ~2.7us ACT_TABLE_LOAD off phase 2's
                # critical path
                warm = nrm.tile([2, 128], F32, name="warm", tag="warm")
                nc.scalar.activation(warm, bsel_sb[0:2, :],
                                     mybir.ActivationFunctionType.Exp)
                def fill_psum():
                    # fillers borrow a score-pool slot (PSUM is fully
                    # subscribed: 3x2 sp banks + 2 otp banks)
                    return sps.tile([128, 2 * QB], F32, name="fp", tag="sp")

                # prime the PE's HAM clock gate during the initial DMA
                # wait: ~36 dummy matmuls on the tiny bsel tile give
                # ~4us of continuous PE activity so the real prologue
                # matmuls run at 2.4GHz instead of 1.2
                pump = fill_psum()
                for _ in range(36):
                    nc.tensor.matmul(pump[:, 0:128], bsel_sb, bsel_sb,
                                     start=True, stop=True)

                def emit_qk_ftile(tb, f):
                    # out rows = 128 cols of wqk f-tile; f0=[q0|q1]->qtab,
                    # f1=[k0|k1]->ktab, f2=[q2|k2]->qt2/kt2 duplicated
                    xc = xc_cur[tb]
                    ps = fill_psum()[:, 0:QB]
                    for e in range(NE):
                        nc.tensor.matmul(ps, wqk_sb[e][:, ts(f, 128)], xc[e],
                                         start=(e == 0), stop=(e == NE - 1))
                    if f == 0:
                        nc.vector.tensor_copy(qtab[tb], ps)
                    elif f == 1:
                        nc.vector.tensor_copy(ktab[tb], ps)
                    else:
                        nc.vector.tensor_copy(qt2[tb][0:64, :], ps[0:64, :])
                        nc.vector.tensor_copy(qt2[tb][64:128, :], ps[0:64, :])
                        nc.vector.tensor_copy(kt2[tb][0:64, :], ps[64:128, :])
                        nc.vector.tensor_copy(kt2[tb][64:128, :], ps[64:128, :])

                def emit_v_stile(tb, st):
                    xc = xc_cur[tb]
                    vp = fill_psum()[:, 0:HPC * D]
                    for e in range(NE):
                        nc.tensor.matmul(vp, xc[e][:, ts(st, 128)], wv_sb[e],
                                         start=(e == 0), stop=(e == NE - 1))
                    for h in range(HPC):
                        nc.vector.tensor_copy(
                            vtok[h][tb][:, st * 65: st * 65 + 64],
                            vp[:, ts(h, D)])
                    if st == 3:
                        del xc_cur[tb]

                def qkv_units(tb):
                    u = [(1300, lambda tb=tb, f=f: emit_qk_ftile(tb, f))
                         for f in range(3)]
                    u += [(700, lambda tb=tb, st=st: emit_v_stile(tb, st))
                          for st in range(4)]
                    return u

                def qk01_units(tb):
                    return [(1300, lambda tb=tb, f=f: emit_qk_ftile(tb, f))
                            for f in range(2)]

                def rest_units(tb):
                    u = [(1300, lambda tb=tb: emit_qk_ftile(tb, 2))]
                    u += [(700, lambda tb=tb, st=st: emit_v_stile(tb, st))
                          for st in range(4)]
                    return u

                def emit_proj(tt):
                    # two 128-token tiles per unit, one batched y DMA
                    yt = ysb.tile([128, 2 * E], F32, name="yt", tag="yt")
                    for k in range(2):
                        Jb = (tt + k) // 4
                        wp2x = wp2e if (Jb % 2 == 0) else wp2o
                        for eh in range(2):
                            pp = fill_psum()[:, 0:E // 2]
                            nc.tensor.matmul(
                                pp, otab[Jb][:, ts((tt + k) % 4, 128)],
                                wpab_sb[:, ts(eh, E // 2)],
                                start=True, stop=False)
                            nc.tensor.matmul(
                                pp, ot2[Jb][:, ts((tt + k) % 4, 128)],
                                wp2x[:, ts(eh, E // 2)],
                                start=False, stop=True)
                            nc.vector.tensor_copy(
                                yt[:, k * E + eh * (E // 2):
                                   k * E + (eh + 1) * (E // 2)], pp)
                    nc.sync.dma_start(
                        out=y[tt * 128:(tt + 2) * 128, :].rearrange(
                            "(t p) c -> p t c", t=2),
                        in_=yt)

                # ------------- phase 2 (attention) emission -------------
                # filler units: [cost_ns, fn, gate, counted]; gate is a
                # tuple of finalize tags that must be emitted first (proj
                # units), or None. Gated-unready units are parked so they
                # never block the qkv force-drains behind them.
                fillers = []
                parked = []
                n_added = [0]
                n_drained = [0]
                marks = {}
                pending_fin = []
                fin_emitted = set()

                def add_fillers(units, mark=None, gate=None):
                    for cost, fn in units:
                        fillers.append([cost, fn, gate, False])
                    n_added[0] += len(units)
                    if mark is not None:
                        marks[mark] = n_added[0]

                def set_mark(mark, back=0):
                    marks[mark] = n_added[0] - back

                def _count(u):
                    if not u[3]:
                        u[3] = True
                        n_drained[0] += 1

                def _take():
                    while fillers:
                        u = fillers.pop(0)
                        _count(u)
                        if u[2] is None or all(t in fin_emitted
                                               for t in u[2]):
                            return u
                        parked.append(u)
                    return None

                def unpark():
                    ready = [u for u in parked
                             if all(t in fin_emitted for t in u[2])]
                    for u in ready:
                        parked.remove(u)
                    fillers[:0] = ready

                def drain_filler(budget):
                    while fillers and budget > 0:
                        u = _take()
                        if u is None:
                            return
                        u[1]()
                        budget -= u[0]

                def drain_all():
                    while fillers:
                        u = _take()
                        if u is not None:
                            u[1]()

                def drain_to(mark):
                    if mark not in marks:
                        return
                    while n_drained[0] < marks[mark] and fillers:
                        u = _take()
                        if u is not None:
                            u[1]()

                def c0_of(J, i):
                    r = i - 4 * J
                    return 0 if r < 0 else 128 * r

                def emit_av_group(h, J, g, otp, ex):
                    imax = 4 * J + 3
                    for u in range(2):
                        i = 2 * g + u
                        r = i - 4 * J
                        c0 = c0_of(J, i)
                        if r >= 0:
                            # zero strictly-future keys in the diagonal
                            # 128x128 sub-block (tri[k,q] = k<=q)
                            nc.vector.tensor_mul(
                                ex[:, QB * u + c0: QB * u + c0 + 128],
                                ex[:, QB * u + c0: QB * u + c0 + 128],
                                tri_sb)
                        # O^T[d, q] (+ row 64 = denominator)
                        nc.tensor.matmul(
                            otp[:, c0:QB],
                            vtok[h][i // 4][:, (i % 4) * 65:(i % 4) * 65 + 65],
                            ex[:, QB * u + c0: QB * (u + 1)],
                            start=(i == 0), stop=(i == imax))

                def s_exp(spec, g):
                    hA, JA, hB, JB, qtX, ktX, dstA, dstB, mark, tag = spec
                    nA, nB = 2 * JA + 2, 2 * JB + 2
                    a, b = g < nA, g < nB
                    spA = sps.tile([128, 2 * QB], F32, name="spA",
                                   tag="sp") if a else None
                    spB = sps.tile([128, 2 * QB], F32, name="spB",
                                   tag="sp") if b else None
                    # interleave A/B per k-tile so row-packed pairs are
                    # adjacent in the PE queue
                    for u in range(2):
                        if a:
                            i = 2 * g + u
                            c0 = c0_of(JA, i)
                            nc.tensor.matmul(
                                spA[:, QB * u + c0: QB * (u + 1)],
                                ktX[i // 4][0:64, ts(i % 4, 128)],
                                qtX[JA][0:64, c0:QB],
                                start=True, stop=True)
                        if b:
                            i = 2 * g + u
                            c0 = c0_of(JB, i)
                            nc.tensor.matmul(
                                spB[:, QB * u + c0: QB * (u + 1)],
                                ktX[i // 4][64:128, ts(i % 4, 128)],
                                qtX[JB][64:128, c0:QB],
                                start=True, stop=True)
                    exA = exB = None
                    # start each exp at the first tile's causal offset: the
                    # masked prefix cols of diagonal groups are never read
                    # by AV, so skipping them is free ACT time
                    if a:
                        stA = c0_of(JA, 2 * g)
                        exA = asb.tile([128, 2 * QB], BF16, name="exA",
                                       tag="ex")
                        nc.scalar.activation(
                            exA[:, stA:], spA[:, stA:],
                            mybir.ActivationFunctionType.Exp)
                    if b:
                        stB = c0_of(JB, 2 * g)
                        exB = asb.tile([128, 2 * QB], BF16, name="exB",
                                       tag="ex")
                        nc.scalar.activation(
                            exB[:, stB:], spB[:, stB:],
                            mybir.ActivationFunctionType.Exp)
                    return exA, exB

                head_ex = [None]

                def emit_pair(spec, next_head):
                    # Cross-pair software pipelining: this pair's first
                    # S/exp group was already emitted inside the previous
                    # pair's last group (head_ex); symmetrically, the next
                    # pair's head is emitted inside our second-to-last
                    # group. AV runs two groups behind S/exp so the
                    # previous pair's finalize broadcast (emitted at our
                    # g=1, after its recb is already computed) never
                    # blocks S matmuls in the PE queue, and its bct slot
                    # WAR resolves instantly.
                    hA, JA, hB, JB, qtX, ktX, dstA, dstB, mark, tag = spec
                    nA, nB = 2 * JA + 2, 2 * JB + 2
                    n = max(nA, nB)
                    if head_ex[0] is None:
                        drain_to(mark)
                        ex_q = [s_exp(spec, 0)]
                    else:
                        ex_q = [head_ex[0]]
                        head_ex[0] = None
                    otpA = otpB = None
                    for g in range(n + 1):
                        if g + 1 < n:
                            ex_q.append(s_exp(spec, g + 1))
                        elif g + 1 == n and next_head is not None:
                            next_head()
                        if g == 1:
                            for fz in pending_fin:
                                fz()
                            pending_fin.clear()
                            # allocate otp only after the previous pair's
                            # finalize reads are emitted (pool WAR tracking
                            # is snapshot-based)
                            otpA = ops.tile([65, QB], F32, name="otpA",
                                            tag="otp")
                            otpB = ops.tile([65, QB], F32, name="otpB",
                                            tag="otp")
                        if g >= 1:
                            drain_filler(1600)
                            # the V tiles this group's AV reads must be
                            # emitted before the AV matmuls (emission order
                            # IS the dependency order)
                            drain_to(f"v{(2 * (g - 1) + 1) // 4}")
                            exA, exB = ex_q.pop(0)
                            if exA is not None:
                                emit_av_group(hA, JA, g - 1, otpA, exA)
                            if exB is not None:
                                emit_av_group(hB, JB, g - 1, otpB, exB)

                    # ---- batched finalize, split in two ----
                    # ACT part now (right behind the last exps in the ACT
                    # queue): 1/den as exp(-ln(den)), both streams at once.
                    # denA lives at partition 0, denB at partition 32
                    # (partition bases must be 32-aligned); other rows are
                    # memset to 1.0 so Ln/Exp stay NaN-free.
                    den = nrm.tile([33, QB], F32, name="den", tag="den")
                    nc.vector.memset(den, 1.0)
                    nc.vector.tensor_copy(den[0:1, :], otpA[64:65, :])
                    nc.vector.tensor_copy(den[32:33, :], otpB[64:65, :])
                    lg = nrm.tile([33, QB], F32, name="lg", tag="lg")
                    nc.scalar.activation(lg, den,
                                         mybir.ActivationFunctionType.Ln)
                    recb = nrm.tile([33, QB], BF16, name="recb", tag="recb")
                    nc.scalar.activation(recb, lg,
                                         mybir.ActivationFunctionType.Exp,
                                         scale=-1.0)

                    # PE/DVE part deferred into the next pair (g=1), by
                    # which point recb is long done: the broadcast matmul
                    # and the normalizing multiplies
                    def finalize():
                        # broadcast: rows 0-63 = 1/denA, 64-127 = 1/denB
                        bct = sps.tile([128, 2 * QB], F32, name="bct", tag="sp")
                        bcp = bct[:, 0:QB]
                        nc.tensor.matmul(bcp, bsel_sb, recb, start=True,
                                         stop=True)
                        bc = nrm.tile([128, QB], F32, name="bc", tag="bc")
                        nc.vector.tensor_copy(bc, bcp)
                        # stream B's O rows move to partitions 64-127 so the
                        # multiply stays base-aligned (DVE tensor_tensor
                        # cannot cross partition bases; copies can)
                        ob = nrm.tile([128, QB], F32, name="ob", tag="ob")
                        nc.vector.tensor_copy(ob[64:128, :], otpB[0:64, :])
                        nc.vector.tensor_mul(dstA[0:64, :], otpA[0:64, :],
                                             bc[0:64, :])
                        nc.vector.tensor_mul(dstB[64:128, :], ob[64:128, :],
                                             bc[64:128, :])
                        fin_emitted.add(tag)
                        unpark()

                    pending_fin.append(finalize)

                # ---------------- schedule ----------------
                def proj_units(Jb):
                    return [(1800, lambda tt=tt: emit_proj(tt))
                            for tt in (4 * Jb, 4 * Jb + 2)]

                def add_qkv(tb):
                    emit_x_dma(tb)
                    add_fillers(qkv_units(tb))
                    set_mark(f"qk{tb}", back=5)   # after f0,f1
                    set_mark(f"f2_{tb}", back=4)  # after f0,f1,f2
                    set_mark(f"v{tb}")            # after all V tiles

                plan = []

                def pair01(J):
                    plan.append(('p', (0, J, 1, J, qtab, ktab, otab[J],
                                       otab[J], f"qk{J}", f"p01_{J}")))

                def pair2(J):
                    plan.append(('p', (2, J, 2, J + 1, qt2, kt2, ot2[J],
                                       ot2[J + 1], f"f2_{J + 1}", f"p2_{J}")))

                def do(fn):
                    plan.append(('d', fn))

                # minimal prologue: only q/k of block 0, so the first
                # exps hit ACT a few us in; everything else is filler
                marks["qk0"] = 0
                for _, u in qk01_units(0):
                    u()
                add_fillers(rest_units(0))
                set_mark("f2_0", back=4)
                set_mark("v0")
                add_fillers(qk01_units(1), mark="qk1")
                add_fillers(rest_units(1))
                set_mark("f2_1", back=4)
                set_mark("v1")

                # pair order interleaves small-J and big-J pairs so the
                # filler supply stays roughly level; qkv lands just-in-time
                pair01(0)
                do(lambda: add_qkv(2))
                pair2(0)
                do(lambda: add_qkv(3))
                pair01(1)
                pair01(2)
                do(lambda: (add_fillers(proj_units(0), gate=("p01_0", "p2_0")),
                            add_fillers(proj_units(1), gate=("p01_1", "p2_0"))))
                pair01(3)
                do(lambda: add_qkv(4))
                pair2(2)
                do(lambda: add_qkv(5))
                pair01(4)
                do(lambda: (add_fillers(proj_units(2), gate=("p01_2", "p2_2")),
                            add_fillers(proj_units(3), gate=("p01_3", "p2_2"))))
                pair2(4)
                do(lambda: add_qkv(6))
                pair01(5)
                do(lambda: (add_fillers(proj_units(4), gate=("p01_4", "p2_4")),
                            add_fillers(proj_units(5), gate=("p01_5", "p2_4"))))
                do(lambda: add_qkv(7))
                pair01(6)
                pair2(6)
                do(lambda: add_fillers(proj_units(6), gate=("p01_6", "p2_6")))
                pair01(7)

                # ---- drive the plan with cross-pair head pipelining ----
                pidx = [i for i, (k, _) in enumerate(plan) if k == 'p']

                def make_head(pi, ni):
                    nspec = plan[ni][1]
                    dos = [v for (k, v) in plan[pi + 1:ni] if k == 'd']

                    def head():
                        for fn in dos:
                            fn()
                        drain_to(nspec[8])
                        head_ex[0] = s_exp(nspec, 0)
                    return head

                for k, v in plan[:pidx[0]]:
                    if k == 'd':
                        v()
                for j, pi in enumerate(pidx):
                    ni = pidx[j + 1] if j + 1 < len(pidx) else None
                    nh = make_head(pi, ni) if ni is not None else None
                    emit_pair(plan[pi][1], nh)
                for k, v in plan[pidx[-1] + 1:]:
                    if k == 'd':
                        v()
                for fz in pending_fin:
                    fz()
                pending_fin.clear()
                drain_all()

                # tail: last q-block's projection
                for tt in (28, 30):
                    emit_proj(tt)

    _split_multi_waits(nc)
    return nc


def _get_nc(with_bias):
    if with_bias not in _nc:
        _nc[with_bias] = _build_program(with_bias)
    return _nc[with_bias]


def _bf16(a):
    return np.ascontiguousarray(a.astype(ml_dtypes.bfloat16))


def ts_(j):
    return slice(j * D, (j + 1) * D)


def kernel(x, W_attn, b_attn, W_proj, b_proj):
    x = np.asarray(x, dtype=np.float32)
    W_attn = np.asarray(W_attn, dtype=np.float32)
    b_attn = np.asarray(b_attn, dtype=np.float32)
    W_proj = np.asarray(W_proj, dtype=np.float32)
    b_proj = np.asarray(b_proj, dtype=np.float32)

    scale = 1.0 / np.sqrt(np.float32(D))
    with_bias = bool(np.any(b_attn != 0.0))
    EA = 832 if with_bias else 768

    # x^T per batch (optionally augmented with a ones row for the bias)
    xT_b = []
    for b in range(B):
        xa = np.zeros((EA, S), dtype=np.float32)
        xa[:E] = x[b].T
        if with_bias:
            xa[E] = 1.0
        xT_b.append(_bf16(xa))

    tri_np = _bf16(np.triu(np.ones((128, 128), dtype=np.float32)))
    bsel_np = np.zeros((33, 128), dtype=np.float32)
    bsel_np[0, 0:64] = 1.0
    bsel_np[32, 64:128] = 1.0
    bsel_np = _bf16(bsel_np)

    in_maps = []
    for c in range(NCORES):
        b = c // 4
        heads = [HPC * (c % 4) + j for j in range(HPC)]
        # wqk cols: [q_h0|q_h1|k_h0|k_h1|q_h2|k_h2]; q pre-scaled by 1/8
        wqk = np.zeros((EA, 2 * HPC * D), dtype=np.float32)
        wv = np.zeros((EA, HPC * D), dtype=np.float32)
        col_q = {0: 0, 1: 1, 2: 4}
        col_k = {0: 2, 1: 3, 2: 5}
        for j, h in enumerate(heads):
            wqk[:E, ts_(col_q[j])] = W_attn[:, h * D:(h + 1) * D] * scale
            wqk[:E, ts_(col_k[j])] = W_attn[:, E + h * D:E + (h + 1) * D]
            wv[:E, ts_(j)] = W_attn[:, 2 * E + h * D:2 * E + (h + 1) * D]
            if with_bias:
                wqk[E, ts_(col_q[j])] = b_attn[h * D:(h + 1) * D] * scale
                wqk[E, ts_(col_k[j])] = b_attn[E + h * D:E + (h + 1) * D]
                wv[E, ts_(j)] = b_attn[2 * E + h * D:2 * E + (h + 1) * D]
        wpab = np.concatenate(
            [W_proj[h * D:(h + 1) * D, :] for h in heads[:2]], axis=0)
        wp2 = W_proj[heads[2] * D:(heads[2] + 1) * D, :]
        in_maps.append({
            "xT": xT_b[b],
            "wqk": _bf16(wqk),
            "wv": _bf16(wv),
            "wpab": _bf16(wpab),
            "wp2": _bf16(wp2),
            "tri": tri_np,
            "bsel": bsel_np,
        })

    nc = _get_nc(with_bias)
    global LAST_EXEC_NS
    if TRACE:
        _install_ntff_hook()
        res = run_bass_kernel_spmd(nc, in_maps, core_ids=list(range(NCORES)),
                                   trace=True)
        LAST_EXEC_NS = res.exec_time_ns
    else:
        res = run_bass_kernel_spmd(nc, in_maps, core_ids=list(range(NCORES)))

    y = np.zeros((B, S, E), dtype=np.float32)
    for c in range(NCORES):
        y[c // 4] += res.results[c]["y"]
    y += b_proj
    return y


def _install_ntff_hook():
    """Register the axon NTFF profiling hook (dev/profiling only)."""
    import sys, types
    try:
        import antenv
        try:
            from antenv.axon_hooks import get_axon_ntff_profile_hook  # noqa
            return
        except ImportError:
            pass
        hooks_mod = types.ModuleType("antenv.axon_hooks")
        _hook = [None]
        hooks_mod.set_axon_ntff_profile_hook = lambda h: _hook.__setitem__(0, h)
        hooks_mod.get_axon_ntff_profile_hook = lambda: _hook[0]
        sys.modules["antenv.axon_hooks"] = hooks_mod
        antenv.axon_hooks = hooks_mod
        from trn_agent_boot.trn_boot import _ntff_profile_via_ctypes
        hooks_mod.set_axon_ntff_profile_hook(
            _ntff_profile_via_ctypes('/opt/axon/libaxon_pjrt.so'))
    except Exception:
        pass


# revision 59
# speedup vs baseline: 1.1556x; 1.0112x over previous
"""Causal self-attention (GPT-2 style) on 8 TRN2 NeuronCores.

Sharding: B=2 x H=12 -> 24 (batch, head) pairs; core c handles batch c//4
and heads [3*(c%4), 3*(c%4)+3). Each core computes QKV for its 3 heads,
causal attention (flash-style, scores^T layout), and a partial output
projection; the host sums the 4 per-batch partials and adds b_proj.

v2: ACT(exp)-bound design. The S^T matmuls run as K=64 row-packed pairs
(tile_position (0,0)/(64,0)) so two streams' score matmuls co-execute on
the PE; QKV and output-projection work is interleaved into the
attention slots as PE filler; softmax finalize is batched per stream
pair (one Ln+Exp on [2,512], one K=2 broadcast matmul via a [2,128]
selector). Per-head Q^T/K^T live in opposite partition halves
(h0/h1 packed; h2 duplicated) which also packs the projection
contraction to a full 128 rows. All persistent intermediates are
per-block tiles (single writer) so fused-phase dependency tracking
stays fine-grained.

Self-contained: builds the Bass program on first call, runs via
run_bass_kernel_spmd on cores 0-7.
"""
import numpy as np
import ml_dtypes

import concourse.bass as bass
import concourse.mybir as mybir
import concourse.tile as tile
from concourse.bass import ts
from concourse.vector_clock import ScopedClock
from concourse.bass_utils import run_bass_kernel_spmd

# ---------------------------------------------------------------------------
# Workaround for the container's walrus build, which rejects any instruction
# carrying more than ONE sync-wait command ("Too many sync wait commands").
# 1) patch the TileContext tail drain to funnel its wait-set through
#    single-wait NOPs on SP; 2) post-pass that moves excess on_wait entries
#    from any instruction onto single-wait NOPs inserted before it on the
#    same engine (engine stalls on the NOPs, then issues the instruction —
#    semantics preserved).
# ---------------------------------------------------------------------------
_WAIT_LIMIT = 1


def _patched_drain_and_barrier(self, tick_clock, wait_clock):
    nc = self.nc
    carrier = nc.sync.nop()
    wait_clock.add_sem_waits(carrier.ins, ScopedClock({None: tick_clock.global_clock}))
    si = carrier.ins.sync_info
    waits = list(si.on_wait) if si and si.on_wait else []
    if len(waits) > _WAIT_LIMIT:
        si.on_wait = waits[:_WAIT_LIMIT]
        for w in waits[_WAIT_LIMIT:]:
            n2 = nc.sync.nop()
            s2 = n2.ins.sync_info
            if s2 is None:
                n2.ins.sync_info = mybir.SyncInfo(on_wait=[w], on_update=[])
            else:
                s2.on_wait = [w]
    nc.sync.drain()
    nc.all_engine_barrier()
    popped = nc._tile_sem_poison_stack.pop()
    assert popped is self._sem_poison
    nc.clear_and_free_semaphores(list(self.sems.allocated().values()))
    nc.all_engine_barrier()


tile.TileContext._drain_and_barrier = _patched_drain_and_barrier


def _split_multi_waits(nc):
    n_inserted = 0
    for fn in nc.m.functions:
        for blk in fn.blocks:
            new_list = []
            changed = False
            for inst in blk.instructions:
                si = getattr(inst, "sync_info", None)
                waits = list(si.on_wait) if (si is not None and si.on_wait) else []
                if len(waits) > _WAIT_LIMIT:
                    extra = waits[: len(waits) - _WAIT_LIMIT]
                    keep = waits[len(waits) - _WAIT_LIMIT:]
                    for w in extra:
                        nop = mybir.InstNoOp(
                            name=f"wsplit-{n_inserted}",
                            sync_info=mybir.SyncInfo(on_wait=[w], on_update=[]),
                            bass_nofuse=True,
                            engine=inst.engine,
                        )
                        new_list.append(nop)
                        n_inserted += 1
                    si.on_wait = keep
                    changed = True
                new_list.append(inst)
            if changed:
                blk.instructions = new_list
    return n_inserted


# ---------------------------------------------------------------------------
# Problem constants (hardcoded per contract).
# ---------------------------------------------------------------------------
B, S, E, H = 2, 4096, 768, 12
D = 64           # head dim
HPC = 3          # heads per core
NCORES = 8
BF16 = mybir.dt.bfloat16
F32 = mybir.dt.float32
QB = 512         # q-block width
NQB = S // QB    # 8
NKT = S // 128   # 32 k-tiles

TRACE = False
LAST_EXEC_NS = None

_nc = {}


def _echunks(with_bias):
    # contraction chunks over the (augmented) feature dim
    ch = [(e * 128, 128) for e in range(6)]
    if with_bias:
        ch.append((768, 64))  # ones/bias row (+ zero padding)
    return ch


def _build_program(with_bias):
    nc = bass.Bass()
    EA = 832 if with_bias else 768
    ech = _echunks(with_bias)
    NE = len(ech)

    xT = nc.dram_tensor("xT", [EA, S], BF16, kind="ExternalInput")
    wqk = nc.dram_tensor("wqk", [EA, 2 * HPC * D], BF16, kind="ExternalInput")
    wv = nc.dram_tensor("wv", [EA, HPC * D], BF16, kind="ExternalInput")
    wpab = nc.dram_tensor("wpab", [128, E], BF16, kind="ExternalInput")
    wp2 = nc.dram_tensor("wp2", [D, E], BF16, kind="ExternalInput")
    tri = nc.dram_tensor("tri", [128, 128], BF16, kind="ExternalInput")
    bsel = nc.dram_tensor("bsel", [33, 128], BF16, kind="ExternalInput")
    y = nc.dram_tensor("y", [S, E], F32, kind="ExternalOutput")

    with tile.TileContext(nc) as tc:
        with (
            tc.tile_pool(name="wpool", bufs=1) as wpool,
            tc.tile_pool(name="per", bufs=1) as per,
            tc.tile_pool(name="xch", bufs=2) as xch,
            tc.tile_pool(name="sps", bufs=3, space="PSUM") as sps,
            tc.tile_pool(name="ops", bufs=2, space="PSUM") as ops,
            tc.tile_pool(name="asb", bufs=8) as asb,
            tc.tile_pool(name="nrm", bufs=2) as nrm,
            tc.tile_pool(name="ysb", bufs=2) as ysb,
        ):
            FQK = 2 * HPC * D
            xc_cur = {}

            def emit_x_dma(tb):
                # one 3D-AP DMA for all six 128-row feature chunks:
                # sync-queue issue costs ~600ns per DMA, so batching the
                # per-block loads 6->1 matters more than transfer shape
                t = xch.tile([128, 6 * QB], BF16, name="xc6", tag="xc6")
                nc.sync.dma_start(
                    out=t,
                    in_=xT[0:768, ts(tb, QB)].rearrange("(e p) c -> p e c",
                                                        e=6))
                xc = [t[:, e * QB:(e + 1) * QB] for e in range(6)]
                if with_bias:
                    tb_t = xch.tile([64, QB], BF16, name="xcb", tag="xcb")
                    nc.sync.dma_start(out=tb_t, in_=xT[768:832, ts(tb, QB)])
                    xc.append(tb_t)
                xc_cur[tb] = xc

            # --- weights to SBUF (critical-path first: wqk, x0, x1) ---
            bsel_sb = wpool.tile([33, 128], BF16, name="bsel_sb")
            nc.sync.dma_start(out=bsel_sb, in_=bsel[:, :])
            wqk_all = wpool.tile([128, 6 * FQK], BF16, name="wqk_all")
            nc.sync.dma_start(
                out=wqk_all,
                in_=wqk[0:768, :].rearrange("(e p) c -> p e c", e=6))
            wqk_sb = [wqk_all[:, e * FQK:(e + 1) * FQK] for e in range(6)]
            emit_x_dma(0)
            emit_x_dma(1)
            wv_all = wpool.tile([128, 6 * HPC * D], BF16, name="wv_all")
            nc.sync.dma_start(
                out=wv_all,
                in_=wv[0:768, :].rearrange("(e p) c -> p e c", e=6))
            wv_sb = [wv_all[:, e * HPC * D:(e + 1) * HPC * D]
                     for e in range(6)]
            if with_bias:
                wqkb = wpool.tile([64, FQK], BF16, name="wqkb")
                nc.sync.dma_start(out=wqkb, in_=wqk[768:832, :])
                wqk_sb.append(wqkb)
                wvb = wpool.tile([64, HPC * D], BF16, name="wvb")
                nc.sync.dma_start(out=wvb, in_=wv[768:832, :])
                wv_sb.append(wvb)
            # packed projection weights: wpab rows 0-63 = W_h0, 64-127 = W_h1
            wpab_sb = wpool.tile([128, E], BF16, name="wpab")
            nc.sync.dma_start(out=wpab_sb, in_=wpab[:, :])
            # h2 parity tiles: even q-blocks use rows 0-63 of ot2, odd 64-127
            wp2e = wpool.tile([128, E], BF16, name="wp2e")
            nc.sync.dma_start(out=wp2e[0:64, :], in_=wp2[:, :])
            nc.gpsimd.memset(wp2e[64:128, :], 0.0)
            wp2o = wpool.tile([128, E], BF16, name="wp2o")
            nc.gpsimd.memset(wp2o[0:64, :], 0.0)
            nc.sync.dma_start(out=wp2o[64:128, :], in_=wp2[:, :])
            tri_sb = wpool.tile([128, 128], BF16, name="tri_sb")
            nc.sync.dma_start(out=tri_sb, in_=tri[:, :])

            # --- persistent intermediates, per 512-token block ---
            # Feature-major Q^T/K^T: h0 in rows 0-63 + h1 in rows 64-127
            # (row-packed score matmuls); h2 duplicated into both halves.
            qtab = [per.tile([128, QB], BF16, name=f"qtab{j}") for j in range(NQB)]
            ktab = [per.tile([128, QB], BF16, name=f"ktab{j}") for j in range(NQB)]
            qt2 = [per.tile([128, QB], BF16, name=f"qt2_{j}") for j in range(NQB)]
            kt2 = [per.tile([128, QB], BF16, name=f"kt2_{j}") for j in range(NQB)]
            # vtok[h][tb]: token-major V with a ones column per k-tile:
            # cols [65s, 65s+64) = V rows of k-tile 4tb+s, col 65s+64 = 1.0
            vtok = [[per.tile([128, 65 * 4], BF16, name=f"vtok{h}_{j}")
                     for j in range(NQB)] for h in range(HPC)]
            for h in range(HPC):
                for j in range(NQB):
                    nc.vector.memset(vtok[h][j], 1.0)
            # normalized O^T per block: otab rows 0-63 = h0, 64-127 = h1;
            # ot2 rows 0-63 valid on even blocks, 64-127 on odd (other
            # half is garbage, masked by wp2e/wp2o zeros).
            otab = [per.tile([128, QB], BF16, name=f"otab{j}") for j in range(NQB)]
            ot2 = [per.tile([128, QB], BF16, name=f"ot2_{j}") for j in range(NQB)]
            for j in range(NQB):
                # the unused parity half must be zeros, not garbage:
                # 0 x NaN = NaN would poison the projection accumulation
                if j % 2 == 0:
                    nc.gpsimd.memset(ot2[j][64:128, :], 0.0)
                else:
                    nc.gpsimd.memset(ot2[j][0:64, :], 0.0)

            if True:
                # ACT spline-table preload: a throwaway exp during the
                # prologue pulls the ~2.7us ACT_TABLE_LOAD off phase 2's
                # critical path
                warm = nrm.tile([2, 128], F32, name="warm", tag="warm")
                nc.scalar.activation(warm, bsel_sb[0:2, :],
                                     mybir.ActivationFunctionType.Exp)
                def fill_psum():
                    # fillers borrow a score-pool slot (PSUM is fully
                    # subscribed: 3x2 sp banks + 2 otp banks)
                    return sps.tile([128, 2 * QB], F32, name="fp", tag="sp")

                # prime the PE's HAM clock gate during the initial DMA
                # wait: ~36 dummy matmuls on the tiny bsel tile give
                # ~4us of continuous PE activity so the real prologue
                # matmuls run at 2.4GHz instead of 1.2
                pump = fill_psum()
                for _ in range(20):
                    nc.tensor.matmul(pump[:, 0:128], bsel_sb, bsel_sb,
                                     start=True, stop=True)

                def emit_qk_ftile(tb, f):
                    # out rows = 128 cols of wqk f-tile; f0=[q0|q1]->qtab,
                    # f1=[k0|k1]->ktab, f2=[q2|k2]->qt2/kt2 duplicated
                    xc = xc_cur[tb]
                    ps = fill_psum()[:, 0:QB]
                    for e in range(NE):
                        nc.tensor.matmul(ps, wqk_sb[e][:, ts(f, 128)], xc[e],
                                         start=(e == 0), stop=(e == NE - 1))
                    if f == 0:
                        nc.vector.tensor_copy(qtab[tb], ps)
                    elif f == 1:
                        nc.vector.tensor_copy(ktab[tb], ps)
                    else:
                        nc.vector.tensor_copy(qt2[tb][0:64, :], ps[0:64, :])
                        nc.vector.tensor_copy(qt2[tb][64:128, :], ps[0:64, :])
                        nc.vector.tensor_copy(kt2[tb][0:64, :], ps[64:128, :])
                        nc.vector.tensor_copy(kt2[tb][64:128, :], ps[64:128, :])

                def emit_v_stile(tb, st):
                    xc = xc_cur[tb]
                    vp = fill_psum()[:, 0:HPC * D]
                    for e in range(NE):
                        nc.tensor.matmul(vp, xc[e][:, ts(st, 128)], wv_sb[e],
                                         start=(e == 0), stop=(e == NE - 1))
                    for h in range(HPC):
                        nc.vector.tensor_copy(
                            vtok[h][tb][:, st * 65: st * 65 + 64],
                            vp[:, ts(h, D)])
                    if st == 3:
                        del xc_cur[tb]

                def qkv_units(tb):
                    u = [(1300, lambda tb=tb, f=f: emit_qk_ftile(tb, f))
                         for f in range(3)]
                    u += [(700, lambda tb=tb, st=st: emit_v_stile(tb, st))
                          for st in range(4)]
                    return u

                def qk01_units(tb):
                    return [(1300, lambda tb=tb, f=f: emit_qk_ftile(tb, f))
                            for f in range(2)]

                def rest_units(tb):
                    u = [(1300, lambda tb=tb: emit_qk_ftile(tb, 2))]
                    u += [(700, lambda tb=tb, st=st: emit_v_stile(tb, st))
                          for st in range(4)]
                    return u

                def emit_proj(tt):
                    # two 128-token tiles per unit, one batched y DMA
                    yt = ysb.tile([128, 2 * E], F32, name="yt", tag="yt")
                    for k in range(2):
                        Jb = (tt + k) // 4
                        wp2x = wp2e if (Jb % 2 == 0) else wp2o
                        for eh in range(2):
                            pp = fill_psum()[:, 0:E // 2]
                            nc.tensor.matmul(
                                pp, otab[Jb][:, ts((tt + k) % 4, 128)],
                                wpab_sb[:, ts(eh, E // 2)],
                                start=True, stop=False)
                            nc.tensor.matmul(
                                pp, ot2[Jb][:, ts((tt + k) % 4, 128)],
                                wp2x[:, ts(eh, E // 2)],
                                start=False, stop=True)
                            nc.vector.tensor_copy(
                                yt[:, k * E + eh * (E // 2):
                                   k * E + (eh + 1) * (E // 2)], pp)
                    nc.sync.dma_start(
                        out=y[tt * 128:(tt + 2) * 128, :].rearrange(
                            "(t p) c -> p t c", t=2),
                        in_=yt)

                # ------------- phase 2 (attention) emission -------------
                # filler units: [cost_ns, fn, gate, counted]; gate is a
                # tuple of finalize tags that must be emitted first (proj
                # units), or None. Gated-unready units are parked so they
                # never block the qkv force-drains behind them.
                fillers = []
                parked = []
                n_added = [0]
                n_drained = [0]
                marks = {}
                pending_fin = []
                fin_emitted = set()

                def add_fillers(units, mark=None, gate=None):
                    for cost, fn in units:
                        fillers.append([cost, fn, gate, False])
                    n_added[0] += len(units)
                    if mark is not None:
                        marks[mark] = n_added[0]

                def set_mark(mark, back=0):
                    marks[mark] = n_added[0] - back

                def _count(u):
                    if not u[3]:
                        u[3] = True
                        n_drained[0] += 1

                def _take():
                    while fillers:
                        u = fillers.pop(0)
                        _count(u)
                        if u[2] is None or all(t in fin_emitted
                                               for t in u[2]):
                            return u
                        parked.append(u)
                    return None

                def unpark():
                    ready = [u for u in parked
                             if all(t in fin_emitted for t in u[2])]
                    for u in ready:
                        parked.remove(u)
                    fillers[:0] = ready

                def drain_filler(budget):
                    while fillers and budget > 0:
                        u = _take()
                        if u is None:
                            return
                        u[1]()
                        budget -= u[0]

                def drain_all():
                    while fillers:
                        u = _take()
                        if u is not None:
                            u[1]()

                def drain_to(mark):
                    if mark not in marks:
                        return
                    while n_drained[0] < marks[mark] and fillers:
                        u = _take()
                        if u is not None:
                            u[1]()

                def c0_of(J, i):
                    r = i - 4 * J
                    return 0 if r < 0 else 128 * r

                def emit_av_group(h, J, g, otp, ex):
                    imax = 4 * J + 3
                    for u in range(2):
                        i = 2 * g + u
                        r = i - 4 * J
                        c0 = c0_of(J, i)
                        if r >= 0:
                            # zero strictly-future keys in the diagonal
                            # 128x128 sub-block (tri[k,q] = k<=q)
                            nc.vector.tensor_mul(
                                ex[:, QB * u + c0: QB * u + c0 + 128],
                                ex[:, QB * u + c0: QB * u + c0 + 128],
                                tri_sb)
                        # O^T[d, q] (+ row 64 = denominator)
                        nc.tensor.matmul(
                            otp[:, c0:QB],
                            vtok[h][i // 4][:, (i % 4) * 65:(i % 4) * 65 + 65],
                            ex[:, QB * u + c0: QB * (u + 1)],
                            start=(i == 0), stop=(i == imax))

                def s_exp(spec, g):
                    hA, JA, hB, JB, qtX, ktX, dstA, dstB, mark, tag = spec
                    nA, nB = 2 * JA + 2, 2 * JB + 2
                    a, b = g < nA, g < nB
                    spA = sps.tile([128, 2 * QB], F32, name="spA",
                                   tag="sp") if a else None
                    spB = sps.tile([128, 2 * QB], F32, name="spB",
                                   tag="sp") if b else None
                    # interleave A/B per k-tile so row-packed pairs are
                    # adjacent in the PE queue
                    for u in range(2):
                        if a:
                            i = 2 * g + u
                            c0 = c0_of(JA, i)
                            nc.tensor.matmul(
                                spA[:, QB * u + c0: QB * (u + 1)],
                                ktX[i // 4][0:64, ts(i % 4, 128)],
                                qtX[JA][0:64, c0:QB],
                                start=True, stop=True)
                        if b:
                            i = 2 * g + u
                            c0 = c0_of(JB, i)
                            nc.tensor.matmul(
                                spB[:, QB * u + c0: QB * (u + 1)],
                                ktX[i // 4][64:128, ts(i % 4, 128)],
                                qtX[JB][64:128, c0:QB],
                                start=True, stop=True)
                    exA = exB = None
                    # start each exp at the first tile's causal offset: the
                    # masked prefix cols of diagonal groups are never read
                    # by AV, so skipping them is free ACT time
                    if a:
                        stA = c0_of(JA, 2 * g)
                        exA = asb.tile([128, 2 * QB], BF16, name="exA",
                                       tag="ex")
                        nc.scalar.activation(
                            exA[:, stA:], spA[:, stA:],
                            mybir.ActivationFunctionType.Exp)
                    if b:
                        stB = c0_of(JB, 2 * g)
                        exB = asb.tile([128, 2 * QB], BF16, name="exB",
                                       tag="ex")
                        nc.scalar.activation(
                            exB[:, stB:], spB[:, stB:],
                            mybir.ActivationFunctionType.Exp)
                    return exA, exB

                head_ex = [None]

                def emit_pair(spec, next_head, next_mark=None):
                    # Cross-pair software pipelining: this pair's first
                    # S/exp group was already emitted inside the previous
                    # pair's last group (head_ex); symmetrically, the next
                    # pair's head is emitted inside our second-to-last
                    # group. AV runs two groups behind S/exp so the
                    # previous pair's finalize broadcast (emitted at our
                    # g=1, after its recb is already computed) never
                    # blocks S matmuls in the PE queue, and its bct slot
                    # WAR resolves instantly.
                    hA, JA, hB, JB, qtX, ktX, dstA, dstB, mark, tag = spec
                    nA, nB = 2 * JA + 2, 2 * JB + 2
                    n = max(nA, nB)
                    if head_ex[0] is None:
                        drain_to(mark)
                        ex_q = [s_exp(spec, 0)]
                    else:
                        ex_q = [head_ex[0]]
                        head_ex[0] = None
                    otpA = otpB = None
                    for g in range(n + 1):
                        if g + 1 < n:
                            ex_q.append(s_exp(spec, g + 1))
                        elif g + 1 == n and next_head is not None:
                            next_head()
                        if g == 1:
                            for fz in pending_fin:
                                fz()
                            pending_fin.clear()
                            # allocate otp only after the previous pair's
                            # finalize reads are emitted (pool WAR tracking
                            # is snapshot-based)
                            otpA = ops.tile([65, QB], F32, name="otpA",
                                            tag="otp")
                            otpB = ops.tile([65, QB], F32, name="otpB",
                                            tag="otp")
                        if g >= 1:
                            budget = 1600
                            if next_mark is not None and g >= n - 4:
                                # catch the filler queue up to the next
                                # pair's prerequisites while our exps still
                                # cover ACT, instead of in a forced burst
                                # at the (ACT-idle) boundary
                                if n_drained[0] < marks.get(next_mark, 0):
                                    budget = 3000
                            drain_filler(budget)
                            # the V tiles this group's AV reads must be
                            # emitted before the AV matmuls (emission order
                            # IS the dependency order)
                            drain_to(f"v{(2 * (g - 1) + 1) // 4}")
                            exA, exB = ex_q.pop(0)
                            if exA is not None:
                                emit_av_group(hA, JA, g - 1, otpA, exA)
                            if exB is not None:
                                emit_av_group(hB, JB, g - 1, otpB, exB)

                    # ---- batched finalize, split in two ----
                    # ACT part now (right behind the last exps in the ACT
                    # queue): 1/den as exp(-ln(den)), both streams at once.
                    # denA lives at partition 0, denB at partition 32
                    # (partition bases must be 32-aligned); other rows are
                    # memset to 1.0 so Ln/Exp stay NaN-free.
                    den = nrm.tile([33, QB], F32, name="den", tag="den")
                    nc.vector.memset(den, 1.0)
                    nc.vector.tensor_copy(den[0:1, :], otpA[64:65, :])
                    nc.vector.tensor_copy(den[32:33, :], otpB[64:65, :])
                    lg = nrm.tile([33, QB], F32, name="lg", tag="lg")
                    nc.scalar.activation(lg, den,
                                         mybir.ActivationFunctionType.Ln)
                    recb = nrm.tile([33, QB], BF16, name="recb", tag="recb")
                    nc.scalar.activation(recb, lg,
                                         mybir.ActivationFunctionType.Exp,
                                         scale=-1.0)

                    # PE/DVE part deferred into the next pair (g=1), by
                    # which point recb is long done: the broadcast matmul
                    # and the normalizing multiplies
                    def finalize():
                        # broadcast: rows 0-63 = 1/denA, 64-127 = 1/denB
                        bct = sps.tile([128, 2 * QB], F32, name="bct", tag="sp")
                        bcp = bct[:, 0:QB]
                        nc.tensor.matmul(bcp, bsel_sb, recb, start=True,
                                         stop=True)
                        bc = nrm.tile([128, QB], F32, name="bc", tag="bc")
                        nc.vector.tensor_copy(bc, bcp)
                        # stream B's O rows move to partitions 64-127 so the
                        # multiply stays base-aligned (DVE tensor_tensor
                        # cannot cross partition bases; copies can)
                        ob = nrm.tile([128, QB], F32, name="ob", tag="ob")
                        nc.vector.tensor_copy(ob[64:128, :], otpB[0:64, :])
                        nc.vector.tensor_mul(dstA[0:64, :], otpA[0:64, :],
                                             bc[0:64, :])
                        nc.vector.tensor_mul(dstB[64:128, :], ob[64:128, :],
                                             bc[64:128, :])
                        fin_emitted.add(tag)
                        unpark()

                    pending_fin.append(finalize)

                # ---------------- schedule ----------------
                def proj_units(Jb):
                    return [(1800, lambda tt=tt: emit_proj(tt))
                            for tt in (4 * Jb, 4 * Jb + 2)]

                def add_qkv(tb):
                    emit_x_dma(tb)
                    add_fillers(qkv_units(tb))
                    set_mark(f"qk{tb}", back=5)   # after f0,f1
                    set_mark(f"f2_{tb}", back=4)  # after f0,f1,f2
                    set_mark(f"v{tb}")            # after all V tiles

                plan = []

                def pair01(J):
                    plan.append(('p', (0, J, 1, J, qtab, ktab, otab[J],
                                       otab[J], f"qk{J}", f"p01_{J}")))

                def pair2(J):
                    plan.append(('p', (2, J, 2, J + 1, qt2, kt2, ot2[J],
                                       ot2[J + 1], f"f2_{J + 1}", f"p2_{J}")))

                def do(fn):
                    plan.append(('d', fn))

                # minimal prologue: only q/k of block 0, so the first
                # exps hit ACT a few us in; everything else is filler
                marks["qk0"] = 0
                for _, u in qk01_units(0):
                    u()
                add_fillers(rest_units(0))
                set_mark("f2_0", back=4)
                set_mark("v0")
                add_fillers(qk01_units(1), mark="qk1")
                add_fillers(rest_units(1))
                set_mark("f2_1", back=4)
                set_mark("v1")

                # pair order interleaves small-J and big-J pairs so the
                # filler supply stays roughly level; qkv lands just-in-time
                pair01(0)
                do(lambda: add_qkv(2))
                pair2(0)
                do(lambda: add_qkv(3))
                pair01(1)
                pair01(2)
                do(lambda: (add_fillers(proj_units(0), gate=("p01_0", "p2_0")),
                            add_fillers(proj_units(1), gate=("p01_1", "p2_0"))))
                pair01(3)
                do(lambda: add_qkv(4))
                pair2(2)
                do(lambda: add_qkv(5))
                pair01(4)
                do(lambda: (add_fillers(proj_units(2), gate=("p01_2", "p2_2")),
                            add_fillers(proj_units(3), gate=("p01_3", "p2_2"))))
                pair2(4)
                do(lambda: add_qkv(6))
                pair01(5)
                do(lambda: (add_fillers(proj_units(4), gate=("p01_4", "p2_4")),
                            add_fillers(proj_units(5), gate=("p01_5", "p2_4"))))
                do(lambda: add_qkv(7))
                pair01(6)
                pair2(6)
                do(lambda: add_fillers(proj_units(6), gate=("p01_6", "p2_6")))
                pair01(7)

                # ---- drive the plan with cross-pair head pipelining ----
                pidx = [i for i, (k, _) in enumerate(plan) if k == 'p']

                def make_head(pi, ni):
                    nspec = plan[ni][1]
                    dos = [v for (k, v) in plan[pi + 1:ni] if k == 'd']

                    def head():
                        for fn in dos:
                            fn()
                        drain_to(nspec[8])
                        head_ex[0] = s_exp(nspec, 0)
                    return head

                for k, v in plan[:pidx[0]]:
                    if k == 'd':
                        v()
                for j, pi in enumerate(pidx):
                    ni = pidx[j + 1] if j + 1 < len(pidx) else None
                    nh = make_head(pi, ni) if ni is not None else None
                    nm = plan[ni][1][8] if ni is not None else None
                    emit_pair(plan[pi][1], nh, nm)
                for k, v in plan[pidx[-1] + 1:]:
                    if k == 'd':
                        v()
                for fz in pending_fin:
                    fz()
                pending_fin.clear()
                drain_all()

                # tail: last q-block's projection
                for tt in (28, 30):
                    emit_proj(tt)

    _split_multi_waits(nc)
    return nc


def _get_nc(with_bias):
    if with_bias not in _nc:
        _nc[with_bias] = _build_program(with_bias)
    return _nc[with_bias]


def _bf16(a):
    return np.ascontiguousarray(a.astype(ml_dtypes.bfloat16))


def ts_(j):
    return slice(j * D, (j + 1) * D)


def kernel(x, W_attn, b_attn, W_proj, b_proj):
    x = np.asarray(x, dtype=np.float32)
    W_attn = np.asarray(W_attn, dtype=np.float32)
    b_attn = np.asarray(b_attn, dtype=np.float32)
    W_proj = np.asarray(W_proj, dtype=np.float32)
    b_proj = np.asarray(b_proj, dtype=np.float32)

    scale = 1.0 / np.sqrt(np.float32(D))
    with_bias = bool(np.any(b_attn != 0.0))
    EA = 832 if with_bias else 768

    # x^T per batch (optionally augmented with a ones row for the bias)
    xT_b = []
    for b in range(B):
        xa = np.zeros((EA, S), dtype=np.float32)
        xa[:E] = x[b].T
        if with_bias:
            xa[E] = 1.0
        xT_b.append(_bf16(xa))

    tri_np = _bf16(np.triu(np.ones((128, 128), dtype=np.float32)))
    bsel_np = np.zeros((33, 128), dtype=np.float32)
    bsel_np[0, 0:64] = 1.0
    bsel_np[32, 64:128] = 1.0
    bsel_np = _bf16(bsel_np)

    in_maps = []
    for c in range(NCORES):
        b = c // 4
        heads = [HPC * (c % 4) + j for j in range(HPC)]
        # wqk cols: [q_h0|q_h1|k_h0|k_h1|q_h2|k_h2]; q pre-scaled by 1/8
        wqk = np.zeros((EA, 2 * HPC * D), dtype=np.float32)
        wv = np.zeros((EA, HPC * D), dtype=np.float32)
        col_q = {0: 0, 1: 1, 2: 4}
        col_k = {0: 2, 1: 3, 2: 5}
        for j, h in enumerate(heads):
            wqk[:E, ts_(col_q[j])] = W_attn[:, h * D:(h + 1) * D] * scale
            wqk[:E, ts_(col_k[j])] = W_attn[:, E + h * D:E + (h + 1) * D]
            wv[:E, ts_(j)] = W_attn[:, 2 * E + h * D:2 * E + (h + 1) * D]
            if with_bias:
                wqk[E, ts_(col_q[j])] = b_attn[h * D:(h + 1) * D] * scale
                wqk[E, ts_(col_k[j])] = b_attn[E + h * D:E + (h + 1) * D]
                wv[E, ts_(j)] = b_attn[2 * E + h * D:2 * E + (h + 1) * D]
        wpab = np.concatenate(
            [W_proj[h * D:(h + 1) * D, :] for h in heads[:2]], axis=0)
        wp2 = W_proj[heads[2] * D:(heads[2] + 1) * D, :]
        in_maps.append({
            "xT": xT_b[b],
            "wqk": _bf16(wqk),
            "wv": _bf16(wv),
            "wpab": _bf16(wpab),
            "wp2": _bf16(wp2),
            "tri": tri_np,
            "bsel": bsel_np,
        })

    nc = _get_nc(with_bias)
    global LAST_EXEC_NS
    if TRACE:
        _install_ntff_hook()
        res = run_bass_kernel_spmd(nc, in_maps, core_ids=list(range(NCORES)),
                                   trace=True)
        LAST_EXEC_NS = res.exec_time_ns
    else:
        res = run_bass_kernel_spmd(nc, in_maps, core_ids=list(range(NCORES)))

    y = np.zeros((B, S, E), dtype=np.float32)
    for c in range(NCORES):
        y[c // 4] += res.results[c]["y"]
    y += b_proj
    return y


def _install_ntff_hook():
    """Register the axon NTFF profiling hook (dev/profiling only)."""
    import sys, types
    try:
        import antenv
        try:
            from antenv.axon_hooks import get_axon_ntff_profile_hook  # noqa
            return
        except ImportError:
            pass
        hooks_mod = types.ModuleType("antenv.axon_hooks")
        _hook = [None]
        hooks_mod.set_axon_ntff_profile_hook = lambda h: _hook.__setitem__(0, h)
        hooks_mod.get_axon_ntff_profile_hook = lambda: _hook[0]
        sys.modules["antenv.axon_hooks"] = hooks_mod
        antenv.axon_hooks = hooks_mod
        from trn_agent_boot.trn_boot import _ntff_profile_via_ctypes
        hooks_mod.set_axon_ntff_profile_hook(
            _ntff_profile_via_ctypes('/opt/axon/libaxon_pjrt.so'))
    except Exception:
        pass
